# revision 1
# baseline (speedup 1.0000x reference)
"""GAT 2-layer message-passing network on 8 TRN2 NeuronCores (Bass/Tile).

v2: dma_gather-based (HW indirect_dma_start only supports 1 idx/partition).

Strategy (dst-sharded):
 - Host: add self loops, sort edges by dst, shard dst-node ranges across cores.
   Each core owns nodes [c*NPC, (c+1)*NPC) and ALL edges into them.
 - Edge slots: per dst-block of 128 nodes, edges sub-grouped by src chunk
   (4 chunks of CH rows so int16 indices work), each (block,chunk) run padded
   to x128 slots = tiles. Superblocks of SBG blocks share gather calls.
 - Phase A (replicated): full feature table htab[n] = [h|a_src|pad] bf16
   [Np, 384] (768B rows for dma_gather), + local stats table stats_loc
   [NPCp, 128] bf16 rows [a_dst(H)|pad] for the core's own nodes (from xT_loc).
 - Phase B (L1): per sb: dma_gather htab rows by src (4 chunk calls) +
   stats_loc rows by local dst (1 call); ex = exp(lrelu(asrc+adst)) batched
   per sb; msg in-place in gather buffer ([h*ex|ex|ex] in cols 0:264);
   one-hot from dloc vs iota; per-block PSUM matmul accumulation over its
   tiles; normalize by summed ex, +b1, relu; h2aug = relu @ W2aug via PE
   transpose; write h2loc (AG input) + h2pad (local gather table).
 - AllGather h2loc -> h2tab [N,4] f32; repack into h2tabp [Npp, 64] f32 rows.
 - Phase C (L2): same slots: gather h2tabp by src (4 chunk calls) + h2pad by
   local dst; 4-wide bf16 messages; one-hot matmuls; normalize, +b2,
   log_softmax -> out [NPC, 2] f32.
"""
import sys

if "/opt/trn_rl_repo" not in sys.path:
    sys.path.insert(0, "/opt/trn_rl_repo")

import math
import numpy as np
import ml_dtypes

import concourse.bass as bass
import concourse.bacc as bacc
import concourse.mybir as mybir
import concourse.tile as tile
from concourse import bass_utils

P = 128
NEG = 0.2
NCHUNK = 4
NQUEUE = 4

# Tile's DMASW sem-lane assignment round-robins over all Pool DMAs, which
# breaks the per-lane FIFO assumption when SWDGE DMAs run on multiple queues
# (out-of-order completion across queues under one counting sem). Patch the
# lane choice to lane == queue_num: per-lane FIFO again holds (each HW ring
# drains in order), and queues get independent lanes.
from concourse import tile_sem_assignment as _tsa  # noqa: E402

if not getattr(_tsa.TileClockTick, "_qaware_patched", False):
    _orig_assign_tick = _tsa.TileClockTick._assign_tick

    def _qaware_assign_tick(self, inst):
        if (isinstance(inst, _tsa.DMAInst)
                and inst.engine == mybir.EngineType.Pool):
            self.next_sw_dma_idx = getattr(inst, "queue_num", 0) or 0
        return _orig_assign_tick(self, inst)

    _tsa.TileClockTick._assign_tick = _qaware_assign_tick
    _tsa.TileClockTick._qaware_patched = True


def _wrap16(flat):
    """[n] -> [128, n//16] wrapped in 16 partitions, replicated x8."""
    w = flat.reshape(-1, 16).T
    return np.tile(w, (8, 1))


# ----------------------------------------------------------------------------
# host-side data prep
# ----------------------------------------------------------------------------

def prep(inputs, cfg):
    N, F, H, C, CLS, NC = cfg["N"], cfg["F"], cfg["H"], cfg["C"], cfg["CLS"], cfg["NC"]
    SBG = cfg.get("SBG", 4)
    x = np.asarray(inputs["x"], np.float32)
    ei = np.asarray(inputs["edge_index"])
    W1 = np.asarray(inputs["W1"], np.float32)
    as1 = np.asarray(inputs["att_src1"], np.float32)
    ad1 = np.asarray(inputs["att_dst1"], np.float32)
    b1 = np.asarray(inputs["b1"], np.float32)
    W2 = np.asarray(inputs["W2"], np.float32)
    as2 = np.asarray(inputs["att_src2"], np.float32)
    ad2 = np.asarray(inputs["att_dst2"], np.float32)
    b2 = np.asarray(inputs["b2"], np.float32)

    HC = H * C
    R1 = HC + 2 * H                      # live row payload [h | asrc | adst]
    RG = 128 * math.ceil(R1 / 128)       # htab gather row elems (bf16, 256B mult)
    NPC = N // NC
    NB = math.ceil(NPC / P)
    NPCp = NB * P                        # padded local rows
    NT = (N + P - 1) // P
    Np = NT * P
    CHB = math.ceil(N / NCHUNK)          # chunk base (same partition L1 & L2)
    assert CHB + (Np - (NCHUNK - 1) * CHB) - CHB < 32768  # last-chunk slice
    assert CHB < 32768 and NPCp < 32768

    # ---- weights / constants -------------------------------------------------
    W1r = W1.reshape(F, H, C)
    Wsrc = np.einsum("fhc,hc->fh", W1r, as1)
    Wdst = np.einsum("fhc,hc->fh", W1r, ad1)
    W1aug = np.concatenate([W1, Wsrc, Wdst], axis=1)          # [F, R1]
    Wsrc2 = W2 @ as2.reshape(CLS, 1)
    Wdst2 = W2 @ ad2.reshape(CLS, 1)
    W2aug = np.concatenate([W2, Wsrc2, Wdst2], axis=1)        # [HC, 4]

    bf16 = ml_dtypes.bfloat16
    xT = np.zeros((F, Np), dtype=bf16)
    xT[:, :N] = x.T.astype(bf16)
    W1aug_b = W1aug.astype(bf16)
    W2aug_b = W2aug.astype(bf16)
    b1rep = np.tile(b1[None, :], (P, 1)).astype(bf16)
    b2rep = np.tile(b2[None, :], (P, 1)).astype(np.float32)
    iota = np.tile(np.arange(P, dtype=np.float32)[None, :], (P, 1)).astype(bf16)
    ident = np.eye(P, dtype=bf16)

    # ---- edges ---------------------------------------------------------------
    src_all = np.concatenate([ei[0], np.arange(N, dtype=ei.dtype)]).astype(np.int64)
    dst_all = np.concatenate([ei[1], np.arange(N, dtype=ei.dtype)]).astype(np.int64)
    order = np.argsort(dst_all, kind="stable")
    src_s = src_all[order]
    dst_s = dst_all[order]
    chunk_s = src_s // CHB

    cnts = np.zeros((NC, NB, NCHUNK), np.int64)
    for c in range(NC):
        for b in range(NB):
            base = c * NPC + b * P
            hi = min(base + P, (c + 1) * NPC)
            lo_i = np.searchsorted(dst_s, base)
            hi_i = np.searchsorted(dst_s, hi)
            ch = chunk_s[lo_i:hi_i]
            for q in range(NCHUNK):
                cnts[c, b, q] = (ch == q).sum()
    Trun = np.ceil(cnts / P).astype(np.int64).max(axis=0)     # [NB, NCHUNK]
    # ensure every block has >= 1 tile total (always true: self loops)

    # superblocks
    sblocks = [list(range(i, min(i + SBG, NB))) for i in range(0, NB, SBG)]
    # slot layout: per sb: for q: for b in sb: Trun[b,q] tiles
    sb_meta = []
    tile_base = 0
    for blist in sblocks:
        segs = []           # per q: (seg_tile_base_global, segT)
        runs = {b: [] for b in blist}   # block -> [(tile_global, T)]
        sb_base = tile_base
        for q in range(NCHUNK):
            segT = int(Trun[blist, q].sum())
            segs.append((tile_base, segT))
            tb = tile_base
            for b in blist:
                t = int(Trun[b, q])
                if t:
                    runs[b].append((tb, t))
                tb += t
            tile_base += segT
        sb_meta.append(dict(base=sb_base, S=tile_base - sb_base, segs=segs,
                            blocks=blist, runs=runs))
    Tsum = tile_base

    # per-core slot-value arrays
    ihsrc_w = np.zeros((NC, P, Tsum * 8), np.int16)
    dloc2d = np.full((NC, P, Tsum), 255.0, np.float32)
    dlocT_a = np.full((NC, 1, Tsum * P), 255.0, np.float32)
    for c in range(NC):
        ihsrc = np.zeros(Tsum * P, np.int16)
        dloc = np.full(Tsum * P, 255.0, np.float32)
        core_lo = np.searchsorted(dst_s, c * NPC)
        core_hi = np.searchsorted(dst_s, (c + 1) * NPC)
        cs, cd, cq = (src_s[core_lo:core_hi], dst_s[core_lo:core_hi],
                      chunk_s[core_lo:core_hi])
        # edges sorted by (dst, chunk); regroup per (block, chunk)
        for sb in sb_meta:
            for q in range(NCHUNK):
                tb = None
                for b in sb["blocks"]:
                    t = int(Trun[b, q])
                    if t == 0:
                        continue
                    # this block+chunk's edges (mask within the dst range)
                    base = c * NPC + b * P
                    hi = min(base + P, (c + 1) * NPC)
                    seg = slice(np.searchsorted(cd, base), np.searchsorted(cd, hi))
                    m = cq[seg] == q
                    es, ed = cs[seg][m], cd[seg][m]
                    n = len(es)
                    assert n <= t * P
                    # locate this run's global tile index (runs are in q order)
                    tg = None
                    for (tgi, tti) in sb["runs"][b]:
                        s0, sT = sb["segs"][q]
                        if s0 <= tgi < s0 + sT:
                            tg = tgi
                            break
                    assert tg is not None
                    s0 = tg * P
                    ihsrc[s0:s0 + n] = (es - q * CHB).astype(np.int16)
                    dloc[s0:s0 + n] = (ed - (c * NPC + b * P)).astype(np.float32)
        ihsrc_w[c] = _wrap16(ihsrc)
        dloc2d[c] = dloc.reshape(Tsum, P).T
        dlocT_a[c, 0] = dloc

    shared = {
        "xT": xT, "W1aug": W1aug_b, "W2aug": W2aug_b, "b1rep": b1rep,
        "b2rep": b2rep, "iota": iota, "ident": ident,
        "iotac": np.arange(P, dtype=np.float32).reshape(P, 1),
        "onesk": np.ones((1, P), np.float32),
    }
    in_maps = []
    for c in range(NC):
        m = dict(shared)
        xl = np.zeros((F, NPCp), dtype=bf16)
        xl[:, :NPC] = xT[:, c * NPC:c * NPC + NPC]
        m["xTloc"] = xl
        m["ihsrc"] = ihsrc_w[c]
        m["dloc2d"] = dloc2d[c]
        m["dlocT"] = dlocT_a[c]
        in_maps.append(m)

    meta = dict(cfg, R1=R1, RG=RG, HC=HC, NPC=NPC, NPCp=NPCp, NB=NB, NT=NT,
                Np=Np, CHB=CHB, Tsum=Tsum, sb_meta=sb_meta, SBG=SBG)
    return in_maps, meta


# ----------------------------------------------------------------------------
# device program
# ----------------------------------------------------------------------------

def _sub(ap, elem_off, dims):
    return bass.AP(ap.tensor, ap.offset + elem_off, [ap.ap[0], *list(dims)])


def build(meta, nc=None):
    N, F, H, C, CLS = meta["N"], meta["F"], meta["H"], meta["C"], meta["CLS"]
    NC, R1, RG, HC = meta["NC"], meta["R1"], meta["RG"], meta["HC"]
    NPC, NPCp, NB, NT, Np = (meta["NPC"], meta["NPCp"], meta["NB"], meta["NT"],
                             meta["Np"])
    CHB, Tsum = meta["CHB"], meta["Tsum"]
    sb_meta = meta["sb_meta"]
    R2 = CLS + 2
    RL2 = 64                           # f32 row elems for L2 gather tables

    f32, bf16, i16 = mybir.dt.float32, mybir.dt.bfloat16, mybir.dt.int16

    if nc is None:
        nc = bacc.Bacc("TRN2", target_bir_lowering=False, debug=False,
                       num_devices=NC, num_swdge_queues=NQUEUE)

    MAXT = 6                 # tiles per dma_gather call (<=768 descs, carveout 1024)
    qrr = [0]

    def gather_split(out_tile, rel, segT, elem, table, ix_tile):
        """Split a segment gather into <=MAXT-tile calls, round-robin queues."""
        done = 0
        while done < segT:
            tt = min(MAXT, segT - done)
            r = rel + done
            nc.gpsimd.dma_gather(
                bass.AP(out_tile[:].tensor, out_tile[:].offset + r * elem,
                        [out_tile[:].ap[0], [elem, tt], [1, elem]]),
                table,
                ix_tile[:, r * 8:(r + tt) * 8],
                tt * P, tt * P, elem,
                queue_num=qrr[0] % NQUEUE,
            )
            qrr[0] += 1
            done += tt

    xT_d = nc.dram_tensor("xT", [F, Np], bf16, kind="ExternalInput")
    xTl_d = nc.dram_tensor("xTloc", [F, NPCp], bf16, kind="ExternalInput")
    W1aug_d = nc.dram_tensor("W1aug", [F, R1], bf16, kind="ExternalInput")
    W2aug_d = nc.dram_tensor("W2aug", [HC, R2], bf16, kind="ExternalInput")
    b1rep_d = nc.dram_tensor("b1rep", [P, HC], bf16, kind="ExternalInput")
    b2rep_d = nc.dram_tensor("b2rep", [P, CLS], f32, kind="ExternalInput")
    iota_d = nc.dram_tensor("iota", [P, P], bf16, kind="ExternalInput")
    ident_d = nc.dram_tensor("ident", [P, P], bf16, kind="ExternalInput")
    ihsrc_d = nc.dram_tensor("ihsrc", [P, Tsum * 8], i16, kind="ExternalInput")
    dloc_d = nc.dram_tensor("dloc2d", [P, Tsum], f32, kind="ExternalInput")
    dlocT_d = nc.dram_tensor("dlocT", [1, Tsum * P], f32, kind="ExternalInput")
    iotac_d = nc.dram_tensor("iotac", [P, 1], f32, kind="ExternalInput")
    onesk_d = nc.dram_tensor("onesk", [1, P], f32, kind="ExternalInput")
    out_d = nc.dram_tensor("out", [NPC, CLS], f32, kind="ExternalOutput")

    htab = nc.dram_tensor("htab", [Np, RG], bf16, kind="Internal")
    sloc = nc.dram_tensor("sloc", [NPCp, H], bf16, kind="Internal")
    h2loc = nc.dram_tensor("h2loc", [NPC, R2], f32, kind="Internal")
    h2pad = nc.dram_tensor("h2pad", [NPCp, R2], f32, kind="Internal")
    h2tab = nc.dram_tensor("h2tab", [N, R2], f32, kind="Internal",
                           addr_space="Shared" if NC > 4 else "Local")
    h2tabp = nc.dram_tensor("h2tabp", [N, RL2], f32, kind="Internal")

    FA = min(P, F)
    FB = F - FA
    NCK = (HC + P - 1) // P

    with tile.TileContext(nc) as tc:
        with tc.tile_pool(name="const", bufs=1) as cp:
            w1a = cp.tile([FA, R1], bf16)
            nc.sync.dma_start(out=w1a[:], in_=W1aug_d[0:FA, :])
            if FB:
                w1b = cp.tile([FB, R1], bf16)
                nc.sync.dma_start(out=w1b[:], in_=W1aug_d[FA:F, :])
            w2s = []
            for k in range(NCK):
                kk = min(P, HC - k * P)
                w2k = cp.tile([kk, R2], bf16, name=f"w2k{k}")
                nc.sync.dma_start(out=w2k[:], in_=W2aug_d[k * P:k * P + kk, :])
                w2s.append(w2k)
            b1s = cp.tile([P, HC], bf16)
            nc.sync.dma_start(out=b1s[:], in_=b1rep_d[:, :])
            b2s = cp.tile([P, CLS], f32)
            nc.sync.dma_start(out=b2s[:], in_=b2rep_d[:, :])
            iot = cp.tile([P, P], bf16)
            nc.sync.dma_start(out=iot[:], in_=iota_d[:, :])
            idn = cp.tile([P, P], bf16)
            nc.sync.dma_start(out=idn[:], in_=ident_d[:, :])
            dlc = cp.tile([P, Tsum], f32)
            nc.sync.dma_start(out=dlc[:], in_=dloc_d[:, :])
            iotc = cp.tile([P, 1], f32)
            nc.sync.dma_start(out=iotc[:], in_=iotac_d[:, :])
            onek = cp.tile([1, P], f32)
            nc.sync.dma_start(out=onek[:], in_=onesk_d[:, :])

            # ---------------- Phase A: feature tables ------------------------
            with tc.tile_pool(name="pa", bufs=3) as pa, \
                 tc.tile_pool(name="psa", bufs=4, space="PSUM") as psa:
                for nt in range(NT):
                    xa = pa.tile([FA, P], bf16)
                    nc.sync.dma_start(out=xa[:], in_=xT_d[0:FA, nt * P:(nt + 1) * P])
                    if FB:
                        xb = pa.tile([FB, P], bf16)
                        nc.sync.dma_start(out=xb[:], in_=xT_d[FA:F, nt * P:(nt + 1) * P])
                    ph = psa.tile([P, R1], f32)
                    nc.tensor.matmul(out=ph[:], lhsT=xa[:], rhs=w1a[:],
                                     start=True, stop=(FB == 0))
                    if FB:
                        nc.tensor.matmul(out=ph[:], lhsT=xb[:], rhs=w1b[:],
                                         start=False, stop=True)
                    hsb = pa.tile([P, R1], bf16)
                    nc.vector.tensor_copy(out=hsb[:], in_=ph[:])
                    nc.sync.dma_start(
                        out=bass.AP(htab, nt * P * RG, [[RG, P], [1, R1]]),
                        in_=hsb[:])
                # local a_dst stats (for the dst-side gather)
                for nt in range(NPCp // P):
                    xa = pa.tile([FA, P], bf16, tag="xla")
                    nc.sync.dma_start(out=xa[:], in_=xTl_d[0:FA, nt * P:(nt + 1) * P])
                    if FB:
                        xb = pa.tile([FB, P], bf16, tag="xlb")
                        nc.sync.dma_start(out=xb[:], in_=xTl_d[FA:F, nt * P:(nt + 1) * P])
                    ps = psa.tile([P, H], f32, tag="pss")
                    nc.tensor.matmul(out=ps[:], lhsT=xa[:],
                                     rhs=w1a[:, HC + H:HC + 2 * H],
                                     start=True, stop=(FB == 0))
                    if FB:
                        nc.tensor.matmul(out=ps[:], lhsT=xb[:],
                                         rhs=w1b[:, HC + H:HC + 2 * H],
                                         start=False, stop=True)
                    ssb = pa.tile([P, H], bf16, tag="ssb")
                    nc.vector.tensor_copy(out=ssb[:], in_=ps[:])
                    nc.sync.dma_start(out=sloc[nt * P:(nt + 1) * P, :], in_=ssb[:])

            # ---------------- Phase B: L1 edge pass --------------------------
            with tc.tile_pool(name="pbg", bufs=2) as pbg, \
                 tc.tile_pool(name="pbb", bufs=2) as pbb, \
                 tc.tile_pool(name="psb", bufs=2, space="PSUM") as psb, \
                 tc.tile_pool(name="pst", bufs=1, space="PSUM") as pst, \
                 tc.tile_pool(name="psh", bufs=1, space="PSUM") as psh, \
                 tc.tile_pool(name="psk", bufs=2, space="PSUM") as psk, \
                 tc.tile_pool(name="psa2", bufs=2, space="PSUM") as psa2:
                for sb in sb_meta:
                    base, S = sb["base"], sb["S"]
                    nblk = len(sb["blocks"])
                    b0 = sb["blocks"][0]
                    g = pbg.tile([P, S * RG], bf16, tag="g")
                    ixs = pbg.tile([P, S * 8], i16, tag="ixs")
                    nc.sync.dma_start(out=ixs[:],
                                      in_=ihsrc_d[:, base * 8:(base + S) * 8])
                    for q in range(NCHUNK):
                        tb, segT = sb["segs"][q]
                        if segT == 0:
                            continue
                        hi = Np if q == NCHUNK - 1 else (q + 1) * CHB
                        gather_split(g, tb - base, segT, RG,
                                     htab[q * CHB:hi, :], ixs)
                    # a_dst window for the sb's blocks  [P, nblk*H] bf16
                    adw = pbg.tile([P, 8 * H], bf16, tag="adw")
                    nc.sync.dma_start(
                        out=adw[:, :nblk * H],
                        in_=bass.AP(sloc, b0 * P * H,
                                    [[H, P], [P * H, nblk], [1, H]]))
                    # O_T: [d, slot] one-hot via PE broadcast of dlocT + is_equal
                    dlT = pbg.tile([1, S * P], f32, tag="dlT")
                    nc.sync.dma_start(out=dlT[:],
                                      in_=dlocT_d[0:1, base * P:(base + S) * P])
                    oT = pbg.tile([P, S * P], bf16, tag="oT", bufs=1)
                    for st in range(0, S * P, 512):
                        w = min(512, S * P - st)
                        stp = psk.tile([P, 512], f32, tag="stp")
                        nc.tensor.matmul(out=stp[:, :w], lhsT=onek[:],
                                         rhs=dlT[0:1, st:st + w],
                                         start=True, stop=True)
                        nc.vector.tensor_tensor(
                            out=oT[:, st:st + w],
                            in0=iotc[:, 0:1].to_broadcast([P, w]),
                            in1=stp[:, :w],
                            op=mybir.AluOpType.is_equal)
                    # per-edge a_dst via O_T matmuls -> PSUM [P, S*H]
                    pad = psa2.tile([P, S * H], f32, tag="pad")
                    for bi, b in enumerate(sb["blocks"]):
                        for (tg, tt) in sb["runs"][b]:
                            for t in range(tt):
                                rel = tg - base + t
                                nc.tensor.matmul(
                                    out=pad[:, rel * H:(rel + 1) * H],
                                    lhsT=oT[:, rel * P:(rel + 1) * P],
                                    rhs=adw[:, bi * H:(bi + 1) * H],
                                    start=True, stop=True,
                                    skip_group_check=True)
                    # ex = exp(lrelu(asrc+adst)) for all slots  [P, S*H] f32
                    ex = pbb.tile([P, S * H], f32, tag="ex", bufs=1)
                    nc.vector.tensor_tensor(
                        out=ex[:].rearrange("p (t h) -> p t h", t=S),
                        in0=_sub(g[:], HC, [[RG, S], [1, H]]),
                        in1=pad[:].rearrange("p (t h) -> p t h", t=S),
                        op=mybir.AluOpType.add)
                    tmp = pbb.tile([P, S * H], f32, tag="tmp", bufs=1)
                    nc.vector.tensor_scalar_mul(out=tmp[:], in0=ex[:], scalar1=NEG)
                    nc.vector.tensor_tensor(out=ex[:], in0=ex[:], in1=tmp[:],
                                            op=mybir.AluOpType.max)
                    nc.scalar.activation(out=ex[:], in_=ex[:],
                                         func=mybir.ActivationFunctionType.Exp)
                    # msg in-place: cols 0:HC *= ex ; cols HC:HC+2H = ex
                    ex3 = ex[:].rearrange("p (t h) -> p t h", t=S)
                    nc.vector.tensor_tensor(
                        out=_sub(g[:], 0, [[RG, S], [C, H], [1, C]]),
                        in0=_sub(g[:], 0, [[RG, S], [C, H], [1, C]]),
                        in1=_sub(ex[:], 0, [[H, S], [1, H], [0, C]]),
                        op=mybir.AluOpType.mult)
                    nc.vector.tensor_copy(out=_sub(g[:], HC, [[RG, S], [1, H]]),
                                          in_=ex3)
                    nc.vector.tensor_copy(out=_sub(g[:], HC + H, [[RG, S], [1, H]]),
                                          in_=ex3)
                    # one-hot for all slots  [P, S*P] bf16
                    oh = pbb.tile([P, S * P], bf16, tag="oh", bufs=1)
                    nc.vector.tensor_tensor(
                        out=oh[:].rearrange("p (t q) -> p t q", t=S),
                        in0=_sub(dlc[:], base, [[1, S], [0, P]]),
                        in1=_sub(iot[:], 0, [[0, S], [1, P]]),
                        op=mybir.AluOpType.is_equal)
                    # per-block accumulation + normalize + L2 prep
                    for b in sb["blocks"]:
                        runs = sb["runs"][b]
                        ntile = sum(t for _, t in runs)
                        pso = psb.tile([P, R1], f32, tag="pso")
                        ti = 0
                        for (tg, tt) in runs:
                            for t in range(tt):
                                rel = tg - base + t
                                nc.tensor.matmul(
                                    out=pso[:],
                                    lhsT=oh[:, rel * P:(rel + 1) * P],
                                    rhs=g[:, rel * RG:rel * RG + R1],
                                    start=(ti == 0), stop=(ti == ntile - 1))
                                ti += 1
                        rows = min(P, NPC - b * P)
                        den = pbb.tile([P, H], f32, tag="den")
                        nc.vector.tensor_scalar_max(out=den[:],
                                                    in0=pso[:, HC:HC + H],
                                                    scalar1=1e-20)
                        rde = pbb.tile([P, H], f32, tag="rde")
                        nc.vector.reciprocal(out=rde[:], in_=den[:])
                        o1 = pbb.tile([P, HC], bf16, tag="o1")
                        for hh in range(H):
                            nc.vector.tensor_scalar_mul(
                                out=o1[:, hh * C:(hh + 1) * C],
                                in0=pso[:, hh * C:(hh + 1) * C],
                                scalar1=rde[:, hh:hh + 1])
                        nc.vector.tensor_tensor(out=o1[:], in0=o1[:], in1=b1s[:],
                                                op=mybir.AluOpType.add)
                        nc.scalar.activation(out=o1[:], in_=o1[:],
                                             func=mybir.ActivationFunctionType.Relu)
                        ph2 = psh.tile([P, R2], f32, tag="ph2")
                        for k in range(NCK):
                            kk = min(P, HC - k * P)
                            ptr = pst.tile([P, P], bf16, tag="ptr")
                            nc.tensor.transpose(out=ptr[:kk, :],
                                                in_=o1[:, k * P:k * P + kk],
                                                identity=idn[:])
                            rT = pbb.tile([P, P], bf16, tag="rT")
                            nc.vector.tensor_copy(out=rT[:kk, :], in_=ptr[:kk, :])
                            nc.tensor.matmul(out=ph2[:], lhsT=rT[:kk, :],
                                             rhs=w2s[k][:kk, :],
                                             start=(k == 0), stop=(k == NCK - 1))
                        h2s = pbb.tile([P, R2], f32, tag="h2s")
                        nc.vector.tensor_copy(out=h2s[:], in_=ph2[:])
                        nc.sync.dma_start(out=h2loc[b * P:b * P + rows, :],
                                          in_=h2s[:rows, :])
                        nc.sync.dma_start(out=h2pad[b * P:(b + 1) * P, :],
                                          in_=h2s[:])

            # ---------------- AllGather + repack -----------------------------
            nc.gpsimd.collective_compute(
                "AllGather", mybir.AluOpType.bypass,
                replica_groups=[list(range(NC))],
                ins=[h2loc[:, :]], outs=[h2tab[:, :]])
            # repack [N, R2] -> 256B rows [N, RL2]
            for r in range(NC):
                nc.sync.dma_start(
                    out=bass.AP(h2tabp, r * NPC * RL2, [[RL2, NPC], [1, R2]]),
                    in_=h2tab[r * NPC:(r + 1) * NPC, :])

            # ---------------- Phase C: L2 edge pass --------------------------
            with tc.tile_pool(name="pcg", bufs=2) as pcg, \
                 tc.tile_pool(name="pcb", bufs=2) as pcb, \
                 tc.tile_pool(name="psc", bufs=2, space="PSUM") as psc, \
                 tc.tile_pool(name="psk2", bufs=2, space="PSUM") as psk2, \
                 tc.tile_pool(name="psd2", bufs=2, space="PSUM") as psd2:
                for sb in sb_meta:
                    base, S = sb["base"], sb["S"]
                    nblk = len(sb["blocks"])
                    b0 = sb["blocks"][0]
                    g2 = pcg.tile([P, S * RL2], f32, tag="g2")
                    ixs = pcg.tile([P, S * 8], i16, tag="ixs2")
                    nc.sync.dma_start(out=ixs[:],
                                      in_=ihsrc_d[:, base * 8:(base + S) * 8])
                    for q in range(NCHUNK):
                        tb, segT = sb["segs"][q]
                        if segT == 0:
                            continue
                        hi = N if q == NCHUNK - 1 else (q + 1) * CHB
                        gather_split(g2, tb - base, segT, RL2,
                                     h2tabp[q * CHB:hi, :], ixs)
                    adw2 = pcg.tile([P, 8], bf16, tag="adw2")
                    nc.gpsimd.dma_start(
                        out=adw2[:, :nblk],
                        in_=bass.AP(h2pad, b0 * P * R2 + CLS + 1,
                                    [[R2, P], [P * R2, nblk], [1, 1]]))
                    dlT = pcg.tile([1, S * P], f32, tag="dlT2")
                    nc.sync.dma_start(out=dlT[:],
                                      in_=dlocT_d[0:1, base * P:(base + S) * P])
                    oT = pcg.tile([P, S * P], bf16, tag="oT2", bufs=1)
                    for st in range(0, S * P, 512):
                        w = min(512, S * P - st)
                        stp = psk2.tile([P, 512], f32, tag="stp2")
                        nc.tensor.matmul(out=stp[:, :w], lhsT=onek[:],
                                         rhs=dlT[0:1, st:st + w],
                                         start=True, stop=True)
                        nc.vector.tensor_tensor(
                            out=oT[:, st:st + w],
                            in0=iotc[:, 0:1].to_broadcast([P, w]),
                            in1=stp[:, :w],
                            op=mybir.AluOpType.is_equal)
                    pad2 = psd2.tile([P, S], f32, tag="pad2")
                    for bi, b in enumerate(sb["blocks"]):
                        for (tg, tt) in sb["runs"][b]:
                            for t in range(tt):
                                rel = tg - base + t
                                nc.tensor.matmul(
                                    out=pad2[:, rel:rel + 1],
                                    lhsT=oT[:, rel * P:(rel + 1) * P],
                                    rhs=adw2[:, bi:bi + 1],
                                    start=True, stop=True,
                                    skip_group_check=True)
                    ex2 = pcb.tile([P, S], f32, tag="ex2")
                    nc.vector.tensor_tensor(
                        out=ex2[:],
                        in0=_sub(g2[:], CLS, [[RL2, S]]),
                        in1=pad2[:],
                        op=mybir.AluOpType.add)
                    tm2 = pcb.tile([P, S], f32, tag="tm2")
                    nc.vector.tensor_scalar_mul(out=tm2[:], in0=ex2[:], scalar1=NEG)
                    nc.vector.tensor_tensor(out=ex2[:], in0=ex2[:], in1=tm2[:],
                                            op=mybir.AluOpType.max)
                    nc.scalar.activation(out=ex2[:], in_=ex2[:],
                                         func=mybir.ActivationFunctionType.Exp)
                    m2 = pcb.tile([P, S * 4], bf16, tag="m2")
                    nc.vector.tensor_copy(out=_sub(m2[:], CLS, [[4, S]]), in_=ex2[:])
                    nc.vector.tensor_copy(out=_sub(m2[:], CLS + 1, [[4, S]]),
                                          in_=ex2[:])
                    nc.vector.tensor_tensor(
                        out=_sub(m2[:], 0, [[4, S], [1, CLS]]),
                        in0=_sub(g2[:], 0, [[RL2, S], [1, CLS]]),
                        in1=_sub(m2[:], CLS, [[4, S], [0, CLS]]),
                        op=mybir.AluOpType.mult)
                    oh2 = pcb.tile([P, S * P], bf16, tag="oh2", bufs=1)
                    nc.vector.tensor_tensor(
                        out=oh2[:].rearrange("p (t q) -> p t q", t=S),
                        in0=_sub(dlc[:], base, [[1, S], [0, P]]),
                        in1=_sub(iot[:], 0, [[0, S], [1, P]]),
                        op=mybir.AluOpType.is_equal)
                    for b in sb["blocks"]:
                        runs = sb["runs"][b]
                        ntile = sum(t for _, t in runs)
                        ps2 = psc.tile([P, 4], f32, tag="ps2")
                        ti = 0
                        for (tg, tt) in runs:
                            for t in range(tt):
                                rel = tg - base + t
                                nc.tensor.matmul(
                                    out=ps2[:],
                                    lhsT=oh2[:, rel * P:(rel + 1) * P],
                                    rhs=m2[:, rel * 4:(rel + 1) * 4],
                                    start=(ti == 0), stop=(ti == ntile - 1))
                                ti += 1
                        rows = min(P, NPC - b * P)
                        den2 = pcb.tile([P, 1], f32, tag="den2")
                        nc.vector.tensor_scalar_max(out=den2[:],
                                                    in0=ps2[:, CLS:CLS + 1],
                                                    scalar1=1e-20)
                        rd2 = pcb.tile([P, 1], f32, tag="rd2")
                        nc.vector.reciprocal(out=rd2[:], in_=den2[:])
                        v = pcb.tile([P, CLS], f32, tag="v")
                        nc.vector.tensor_scalar_mul(out=v[:], in0=ps2[:, 0:CLS],
                                                    scalar1=rd2[:, 0:1])
                        nc.vector.tensor_tensor(out=v[:], in0=v[:], in1=b2s[:],
                                                op=mybir.AluOpType.add)
                        mx = pcb.tile([P, 1], f32, tag="mx")
                        nc.vector.tensor_reduce(out=mx[:], in_=v[:],
                                                axis=mybir.AxisListType.X,
                                                op=mybir.AluOpType.max)
                        u = pcb.tile([P, CLS], f32, tag="u")
                        nc.vector.tensor_scalar_sub(out=u[:], in0=v[:],
                                                    scalar1=mx[:, 0:1])
                        nc.scalar.activation(out=u[:], in_=u[:],
                                             func=mybir.ActivationFunctionType.Exp)
                        sm = pcb.tile([P, 1], f32, tag="sm")
                        nc.vector.tensor_reduce(out=sm[:], in_=u[:],
                                                axis=mybir.AxisListType.X,
                                                op=mybir.AluOpType.add)
                        ls = pcb.tile([P, 1], f32, tag="ls")
                        nc.scalar.activation(out=ls[:], in_=sm[:],
                                             func=mybir.ActivationFunctionType.Ln)
                        nc.vector.tensor_tensor(out=ls[:], in0=ls[:], in1=mx[:],
                                                op=mybir.AluOpType.add)
                        res = pcb.tile([P, CLS], f32, tag="res")
                        nc.vector.tensor_scalar_sub(out=res[:], in0=v[:],
                                                    scalar1=ls[:, 0:1])
                        nc.sync.dma_start(out=out_d[b * P:b * P + rows, :],
                                          in_=res[:rows, :])
    nc.finalize()
    return nc


def install_ntff_hook(so_path="/opt/axon/libaxon_pjrt.so"):
    import types
    import ctypes
    import contextlib
    import antenv

    if getattr(antenv, "axon_hooks", None) is not None:
        return
    lib = ctypes.CDLL(so_path)
    if not hasattr(lib, "axon_start_nrt_profile"):
        return
    lib.axon_start_nrt_profile.argtypes = [ctypes.POINTER(ctypes.c_int64),
                                           ctypes.c_size_t]
    lib.axon_start_nrt_profile.restype = ctypes.c_int64
    lib.axon_stop_nrt_profile.argtypes = [ctypes.c_char_p]
    lib.axon_stop_nrt_profile.restype = ctypes.c_int64

    @contextlib.contextmanager
    def _hook(output_dir, device_ids):
        import jax
        jax.devices()
        if device_ids:
            ids = (ctypes.c_int64 * len(device_ids))(*device_ids)
            rc = lib.axon_start_nrt_profile(ids, len(device_ids))
        else:
            rc = lib.axon_start_nrt_profile(None, 0)
        if rc != 0:
            raise RuntimeError(f"axon_start_nrt_profile rc={rc}")
        try:
            yield
        finally:
            n = lib.axon_stop_nrt_profile(str(output_dir).encode())
            print(f"ntff profile: {n} file(s) written to {output_dir}")

    mod = types.ModuleType("antenv.axon_hooks")
    _reg = [_hook]
    mod.set_axon_ntff_profile_hook = lambda h: _reg.__setitem__(0, h)
    mod.get_axon_ntff_profile_hook = lambda: _reg[0]
    sys.modules["antenv.axon_hooks"] = mod
    antenv.axon_hooks = mod


def run(inputs, cfg, trace=False, **kwargs):
    if trace:
        install_ntff_hook()
    in_maps, meta = prep(inputs, cfg)
    nc = build(meta)
    res = bass_utils.run_bass_kernel_spmd(
        nc, in_maps, core_ids=list(range(cfg["NC"])), trace=trace, **kwargs)
    out = np.concatenate([res.results[c]["out"] for c in range(cfg["NC"])], axis=0)
    return out, res


# ----------------------------------------------------------------------------
# harness entry point
# ----------------------------------------------------------------------------

_CFG = dict(N=100000, F=165, H=4, C=64, CLS=2, NC=8)


def kernel(**inputs):
    """Full (unsharded) inputs -> full [N, 2] float32 log-softmax output.

    Shards edges by destination-node range across the 8 NeuronCores,
    compiles and runs the Bass/Tile kernel via run_bass_kernel_spmd,
    and concatenates the per-core output slices.
    """
    out, _ = run(inputs, _CFG, trace=False)
    return np.ascontiguousarray(out.astype(np.float32))



# revision 8
# speedup vs baseline: 2.0254x; 2.0254x over previous
"""GAT 2-layer message-passing network on 8 TRN2 NeuronCores (Bass/Tile).

v3: gather-free L1 via host-side edge-slot materialization of x.

Strategy (dst-sharded, dense slot layout):
 - Host: add self loops, sort edges by dst, shard dst-node ranges across
   cores (core c owns nodes [c*NPC, (c+1)*NPC) and ALL edges into them).
 - Slots: per superblock of SBG=3 dst-blocks, each core's edges packed
   DENSELY into Tsb = ceil(max_core_count/128) tiles (common layout across
   cores; only trailing padding). A tile touches at most 2 dst blocks
   (maxU=2): host assigns each slot to one-hot set A or B.
 - L1 (Phase B): host pre-gathers x rows into slot order (xgaT [165, SLOTS]
   bf16). Device: h_slot = xga @ [W1|Wsrc] per tile (PE), a_dst via
   oT-matmul against a local per-block table, ex = exp(lrelu(asrc+adst)),
   msg = [h*ex | ex], scatter-add per block via one-hot matmuls, normalize,
   +b1, relu, then h2 = o1 @ [W2|Wsrc2|Wdst2] -> h2c [P, NB*4].
 - AllGather h2loc [NPC,4] f32 -> h2tab [N,4] f32 (= gather table viewed as
   [N/16, 64]: 16 nodes per 256B row; NO repack needed).
 - L2 (Phase C): same slots: dma_gather h2tab16 rows by src//16 (single
   chunk, int16), on-chip lane extract by src%16, ex2 via oT2/adst2 table,
   4-wide messages, one-hot scatter, normalize; batched log_softmax at end.
"""
import sys

if "/opt/trn_rl_repo" not in sys.path:
    sys.path.insert(0, "/opt/trn_rl_repo")

import math
import numpy as np
import ml_dtypes

import concourse.bass as bass
import concourse.bacc as bacc
import concourse.mybir as mybir
import concourse.tile as tile
from concourse import bass_utils

P = 128
NEG = 0.2
NQUEUE = 4
SBG = 3
MAXT = 6

# Tile's DMASW sem-lane assignment round-robins over all Pool DMAs, which
# breaks the per-lane FIFO assumption when SWDGE DMAs run on multiple queues
# (out-of-order completion across queues under one counting sem). Patch the
# lane choice to lane == queue_num: per-lane FIFO again holds (each HW ring
# drains in order), and queues get independent lanes.
from concourse import tile_sem_assignment as _tsa  # noqa: E402

if not getattr(_tsa.TileClockTick, "_qaware_patched", False):
    _orig_assign_tick = _tsa.TileClockTick._assign_tick

    def _qaware_assign_tick(self, inst):
        if (isinstance(inst, _tsa.DMAInst)
                and inst.engine == mybir.EngineType.Pool):
            self.next_sw_dma_idx = getattr(inst, "queue_num", 0) or 0
        return _orig_assign_tick(self, inst)

    _tsa.TileClockTick._assign_tick = _qaware_assign_tick
    _tsa.TileClockTick._qaware_patched = True


def _wrap16(flat):
    """[n] -> [128, n//16] wrapped in 16 partitions, replicated x8."""
    w = flat.reshape(-1, 16).T
    return np.tile(w, (8, 1))


# ----------------------------------------------------------------------------
# host-side data prep
# ----------------------------------------------------------------------------

def prep(inputs, cfg):
    N, F, H, C, CLS, NC = cfg["N"], cfg["F"], cfg["H"], cfg["C"], cfg["CLS"], cfg["NC"]
    x = np.asarray(inputs["x"], np.float32)
    ei = np.asarray(inputs["edge_index"])
    W1 = np.asarray(inputs["W1"], np.float32)
    as1 = np.asarray(inputs["att_src1"], np.float32)
    ad1 = np.asarray(inputs["att_dst1"], np.float32)
    b1 = np.asarray(inputs["b1"], np.float32)
    W2 = np.asarray(inputs["W2"], np.float32)
    as2 = np.asarray(inputs["att_src2"], np.float32)
    ad2 = np.asarray(inputs["att_dst2"], np.float32)
    b2 = np.asarray(inputs["b2"], np.float32)

    HC = H * C                          # 256
    R1 = HC + H                         # 260 = [h | asrc]
    NPC = N // NC
    NB = math.ceil(NPC / P)
    NPCp = NB * P
    bf16 = ml_dtypes.bfloat16

    # ---- weights / constants -------------------------------------------------
    W1r = W1.reshape(F, H, C)
    Wsrc = np.einsum("fhc,hc->fh", W1r, as1)
    Wdst = np.einsum("fhc,hc->fh", W1r, ad1)
    W1s = np.concatenate([W1, Wsrc], axis=1).astype(bf16)     # [F, R1]
    Wd = Wdst.astype(bf16)                                    # [F, H]
    Wsrc2 = W2 @ as2.reshape(CLS, 1)
    Wdst2 = W2 @ ad2.reshape(CLS, 1)
    W2aug = np.concatenate([W2, Wsrc2, Wdst2], axis=1).astype(bf16)  # [HC, 4]

    b1rep = np.tile(b1[None, :], (P, 1)).astype(bf16)
    b2rep = np.tile(b2[None, :], (P, 1)).astype(np.float32)
    iota = np.tile(np.arange(P, dtype=np.float32)[None, :], (P, 1)).astype(bf16)
    ident = np.eye(P, dtype=bf16)
    iotac = np.arange(P, dtype=np.float32).reshape(P, 1)
    onek = np.ones((1, P), dtype=bf16)
    iota16 = np.tile(np.arange(16, dtype=np.float32)[None, :], (P, 1))

    # ---- edges ---------------------------------------------------------------
    src_all = np.concatenate([ei[0], np.arange(N, dtype=ei.dtype)]).astype(np.int64)
    dst_all = np.concatenate([ei[1], np.arange(N, dtype=ei.dtype)]).astype(np.int64)
    order = np.argsort(dst_all, kind="stable")
    src_s = src_all[order]
    dst_s = dst_all[order]

    # per (core, block) counts
    cnt = np.zeros((NC, NB), np.int64)
    for c in range(NC):
        for b in range(NB):
            base = c * NPC + b * P
            hi = min(base + P, (c + 1) * NPC)
            cnt[c, b] = (np.searchsorted(dst_s, hi) -
                         np.searchsorted(dst_s, base))

    # superblock metadata (common across cores)
    sbs = []
    tile_base = 0
    nsb = math.ceil(NB / SBG)
    for s in range(nsb):
        blo, bhi = s * SBG, min((s + 1) * SBG, NB)
        persb = cnt[:, blo:bhi]
        Tsb = int(np.ceil(persb.sum(axis=1).max() / P))
        t0 = np.zeros(bhi - blo, np.int64)
        t1 = np.zeros(bhi - blo, np.int64)
        for bi in range(bhi - blo):
            lo = persb[:, :bi].sum(axis=1)
            hi2 = persb[:, :bi + 1].sum(axis=1)
            t0[bi] = lo.min() // P
            t1[bi] = min(int(np.ceil(hi2 / P).max()) - 1, Tsb - 1)
        # per tile: ordered union of touched blocks (local block index)
        uA = np.full(Tsb, -1, np.int64)
        uB = np.full(Tsb, -1, np.int64)
        for t in range(Tsb):
            u = [bi for bi in range(bhi - blo) if t0[bi] <= t <= t1[bi]]
            assert 1 <= len(u) <= 2, (s, t, u)
            uA[t] = u[0]
            if len(u) == 2:
                uB[t] = u[1]
        sbs.append(dict(base=tile_base, S=Tsb, blo=blo, bhi=bhi,
                        t0=t0, t1=t1, uA=uA, uB=uB))
        tile_base += Tsb
    Tsum = tile_base
    SLOT = Tsum * P
    S_MAX = max(sb["S"] for sb in sbs)

    # ---- per-core slot arrays ------------------------------------------------
    x_bf = x.astype(bf16)
    in_maps = []
    shared = {
        "w1s_a": np.ascontiguousarray(W1s[:P]),
        "w1s_b": np.ascontiguousarray(W1s[P:]),
        "wd_a": np.ascontiguousarray(Wd[:P]),
        "wd_b": np.ascontiguousarray(Wd[P:]),
        "w2aug": W2aug, "b1rep": b1rep, "b2rep": b2rep,
        "iota": iota, "ident": ident, "iotac": iotac, "onek": onek,
        "iota16": iota16,
    }
    for c in range(NC):
        srcslot = np.zeros(SLOT, np.int64)
        dlocA = np.full(SLOT, 255.0, np.float32)
        dlocB = np.full(SLOT, 255.0, np.float32)
        core_lo = np.searchsorted(dst_s, c * NPC)
        core_hi = np.searchsorted(dst_s, (c + 1) * NPC)
        cs = src_s[core_lo:core_hi]
        cd = dst_s[core_lo:core_hi]
        for sb in sbs:
            base_n = c * NPC + sb["blo"] * P
            hi_n = min(c * NPC + sb["bhi"] * P, (c + 1) * NPC)
            lo_i = np.searchsorted(cd, base_n)
            hi_i = np.searchsorted(cd, hi_n)
            es = cs[lo_i:hi_i]
            ed = cd[lo_i:hi_i]
            n = len(es)
            assert n <= sb["S"] * P
            s0 = sb["base"] * P
            srcslot[s0:s0 + n] = es
            # block-local dst and A/B role per slot
            pos = np.arange(n)
            trel = pos // P
            bloc = (ed - c * NPC) // P - sb["blo"]          # local block idx
            dv = (ed - c * NPC - (bloc + sb["blo"]) * P).astype(np.float32)
            isA = sb["uA"][trel] == bloc
            isB = sb["uB"][trel] == bloc
            assert np.all(isA | isB), (c, sb["base"])
            ia = s0 + pos[isA]
            ib = s0 + pos[isB]
            dlocA[ia] = dv[isA]
            dlocB[ib] = dv[isB]
        m = dict(shared)
        xga = np.ascontiguousarray(x_bf[srcslot].T)         # [F, SLOT]
        m["xga_a"] = np.ascontiguousarray(xga[:P])
        m["xga_b"] = np.ascontiguousarray(xga[P:])
        xl = np.zeros((F, NPCp), dtype=bf16)
        xl[:, :NPC] = x_bf[c * NPC:(c + 1) * NPC].T
        m["xtl_a"] = np.ascontiguousarray(xl[:P])
        m["xtl_b"] = np.ascontiguousarray(xl[P:])
        m["ihsrc16"] = _wrap16((srcslot // 16).astype(np.int16))
        m["lsel"] = (srcslot % 16).astype(bf16).reshape(Tsum, P).T.copy()
        m["dlA2"] = dlocA.astype(bf16).reshape(Tsum, P).T.copy()
        m["dlB2"] = dlocB.astype(bf16).reshape(Tsum, P).T.copy()
        m["dlT2"] = np.stack([dlocA.astype(bf16), dlocB.astype(bf16)])
        in_maps.append(m)

    meta = dict(cfg, R1=R1, HC=HC, NPC=NPC, NPCp=NPCp, NB=NB, Tsum=Tsum,
                SLOT=SLOT, S_MAX=S_MAX, sbs=sbs)
    return in_maps, meta


# ----------------------------------------------------------------------------
# device program
# ----------------------------------------------------------------------------

def _sub(ap, elem_off, dims):
    return bass.AP(ap.tensor, ap.offset + elem_off, [ap.ap[0], *list(dims)])


def build(meta, nc=None):
    N, F, H, C, CLS = meta["N"], meta["F"], meta["H"], meta["C"], meta["CLS"]
    NC, R1, HC = meta["NC"], meta["R1"], meta["HC"]
    NPC, NPCp, NB = meta["NPC"], meta["NPCp"], meta["NB"]
    Tsum, SLOT = meta["Tsum"], meta["SLOT"]
    sbs = meta["sbs"]
    FB = F - P                           # 37
    NT16 = N // 16                       # h2tab rows of 16 nodes
    RL2 = 64                             # f32 elems per 256B gather row

    f32, bf16, i16 = mybir.dt.float32, mybir.dt.bfloat16, mybir.dt.int16
    EXP = mybir.ActivationFunctionType.Exp
    LN = mybir.ActivationFunctionType.Ln

    if nc is None:
        nc = bacc.Bacc("TRN2", target_bir_lowering=False, debug=False,
                       num_devices=NC, num_swdge_queues=NQUEUE)

    qrr = [0]

    def gather_split(out_tile, rel, segT, elem, table, ix_tile):
        done = 0
        while done < segT:
            tt = min(MAXT, segT - done)
            r = rel + done
            nc.gpsimd.dma_gather(
                bass.AP(out_tile[:].tensor, out_tile[:].offset + r * elem,
                        [out_tile[:].ap[0], [elem, tt], [1, elem]]),
                table,
                ix_tile[:, r * 8:(r + tt) * 8],
                tt * P, tt * P, elem,
                queue_num=qrr[0] % NQUEUE,
            )
            qrr[0] += 1
            done += tt

    xga_a_d = nc.dram_tensor("xga_a", [P, SLOT], bf16, kind="ExternalInput")
    xga_b_d = nc.dram_tensor("xga_b", [FB, SLOT], bf16, kind="ExternalInput")
    xtl_a_d = nc.dram_tensor("xtl_a", [P, NPCp], bf16, kind="ExternalInput")
    xtl_b_d = nc.dram_tensor("xtl_b", [FB, NPCp], bf16, kind="ExternalInput")
    w1s_a_d = nc.dram_tensor("w1s_a", [P, R1], bf16, kind="ExternalInput")
    w1s_b_d = nc.dram_tensor("w1s_b", [FB, R1], bf16, kind="ExternalInput")
    wd_a_d = nc.dram_tensor("wd_a", [P, H], bf16, kind="ExternalInput")
    wd_b_d = nc.dram_tensor("wd_b", [FB, H], bf16, kind="ExternalInput")
    w2aug_d = nc.dram_tensor("w2aug", [HC, 4], bf16, kind="ExternalInput")
    b1rep_d = nc.dram_tensor("b1rep", [P, HC], bf16, kind="ExternalInput")
    b2rep_d = nc.dram_tensor("b2rep", [P, CLS], f32, kind="ExternalInput")
    iota_d = nc.dram_tensor("iota", [P, P], bf16, kind="ExternalInput")
    ident_d = nc.dram_tensor("ident", [P, P], bf16, kind="ExternalInput")
    iotac_d = nc.dram_tensor("iotac", [P, 1], f32, kind="ExternalInput")
    onek_d = nc.dram_tensor("onek", [1, P], bf16, kind="ExternalInput")
    iota16_d = nc.dram_tensor("iota16", [P, 16], f32, kind="ExternalInput")
    ihsrc_d = nc.dram_tensor("ihsrc16", [P, Tsum * 8], i16, kind="ExternalInput")
    lsel_d = nc.dram_tensor("lsel", [P, Tsum], bf16, kind="ExternalInput")
    dlA2_d = nc.dram_tensor("dlA2", [P, Tsum], bf16, kind="ExternalInput")
    dlB2_d = nc.dram_tensor("dlB2", [P, Tsum], bf16, kind="ExternalInput")
    dlT2_d = nc.dram_tensor("dlT2", [2, SLOT], bf16, kind="ExternalInput")
    out_d = nc.dram_tensor("out", [NPC, CLS], f32, kind="ExternalOutput")

    h2loc = nc.dram_tensor("h2loc", [NPC, 4], f32, kind="Internal")
    h2tab = nc.dram_tensor("h2tab", [N, 4], f32, kind="Internal",
                           addr_space="Shared" if NC > 4 else "Local")

    with tile.TileContext(nc) as tc:
        with tc.tile_pool(name="const", bufs=1) as cp:
            w1sa = cp.tile([P, R1], bf16)
            nc.sync.dma_start(out=w1sa[:], in_=w1s_a_d[:, :])
            w1sb = cp.tile([FB, R1], bf16)
            nc.sync.dma_start(out=w1sb[:], in_=w1s_b_d[:, :])
            w2s = []
            for k in range(2):
                w2k = cp.tile([P, 4], bf16, name=f"w2k{k}")
                nc.sync.dma_start(out=w2k[:], in_=w2aug_d[k * P:(k + 1) * P, :])
                w2s.append(w2k)
            b1s = cp.tile([P, HC], bf16)
            nc.sync.dma_start(out=b1s[:], in_=b1rep_d[:, :])
            b2s = cp.tile([P, CLS], f32)
            nc.sync.dma_start(out=b2s[:], in_=b2rep_d[:, :])
            iot = cp.tile([P, P], bf16)
            nc.sync.dma_start(out=iot[:], in_=iota_d[:, :])
            idn = cp.tile([P, P], bf16)
            nc.sync.dma_start(out=idn[:], in_=ident_d[:, :])
            iotc = cp.tile([P, 1], f32)
            nc.sync.dma_start(out=iotc[:], in_=iotac_d[:, :])
            onk = cp.tile([1, P], bf16)
            nc.sync.dma_start(out=onk[:], in_=onek_d[:, :])
            io16 = cp.tile([P, 16], f32)
            nc.sync.dma_start(out=io16[:], in_=iota16_d[:, :])
            ixs = cp.tile([P, Tsum * 8], i16)
            nc.sync.dma_start(out=ixs[:], in_=ihsrc_d[:, :])
            lsl = cp.tile([P, Tsum], bf16)
            nc.sync.dma_start(out=lsl[:], in_=lsel_d[:, :])
            dlA = cp.tile([P, Tsum], bf16)
            nc.sync.dma_start(out=dlA[:], in_=dlA2_d[:, :])
            dlB = cp.tile([P, Tsum], bf16)
            nc.sync.dma_start(out=dlB[:], in_=dlB2_d[:, :])
            adw = cp.tile([P, NB * H], bf16)      # a_dst1 per local node
            ad2w = cp.tile([P, NB], bf16)         # a_dst2 per local node
            h2c = cp.tile([P, NB * 4], f32)       # h2 collect [out2|asrc2|adst2]
            vcol = cp.tile([P, NB * CLS], f32)    # L2 logits collect

            # ---------------- Phase A-mini: local a_dst1 table ---------------
            with tc.tile_pool(name="pa", bufs=1) as pa, \
                 tc.tile_pool(name="psa", bufs=4, space="PSUM") as psa:
                xla = pa.tile([P, NPCp], bf16)
                nc.sync.dma_start(out=xla[:], in_=xtl_a_d[:, :])
                xlb = pa.tile([FB, NPCp], bf16)
                nc.sync.dma_start(out=xlb[:], in_=xtl_b_d[:, :])
                wda = pa.tile([P, H], bf16)
                nc.sync.dma_start(out=wda[:], in_=wd_a_d[:, :])
                wdb = pa.tile([FB, H], bf16)
                nc.sync.dma_start(out=wdb[:], in_=wd_b_d[:, :])
                for nt in range(NB):
                    ps = psa.tile([P, H], f32, tag="ps")
                    nc.tensor.matmul(out=ps[:], lhsT=xla[:, nt * P:(nt + 1) * P],
                                     rhs=wda[:], start=True, stop=False)
                    nc.tensor.matmul(out=ps[:], lhsT=xlb[:, nt * P:(nt + 1) * P],
                                     rhs=wdb[:], start=False, stop=True)
                    nc.vector.tensor_copy(out=adw[:, nt * H:(nt + 1) * H],
                                          in_=ps[:])

            # ---------------- Phase B: L1 edge pass --------------------------
            with tc.tile_pool(name="pbg", bufs=2) as pbg, \
                 tc.tile_pool(name="pbo", bufs=1) as pbo, \
                 tc.tile_pool(name="pbb", bufs=2) as pbb, \
                 tc.tile_pool(name="psh", bufs=2, space="PSUM") as psh, \
                 tc.tile_pool(name="psk", bufs=1, space="PSUM") as psk, \
                 tc.tile_pool(name="psb", bufs=3, space="PSUM") as psb, \
                 tc.tile_pool(name="pst", bufs=1, space="PSUM") as pst, \
                 tc.tile_pool(name="ps2", bufs=1, space="PSUM") as ps2p:
                for sb in sbs:
                    base, S = sb["base"], sb["S"]
                    blo = sb["blo"]
                    nblk = sb["bhi"] - blo
                    has_b = bool((sb["uB"] >= 0).any())
                    xa = pbg.tile([P, S * P], bf16, tag="xa")
                    nc.sync.dma_start(out=xa[:],
                                      in_=xga_a_d[:, base * P:(base + S) * P])
                    xb = pbg.tile([FB, S * P], bf16, tag="xb")
                    nc.sync.dma_start(out=xb[:],
                                      in_=xga_b_d[:, base * P:(base + S) * P])
                    # h per slot -> psum -> g (bf16)
                    g = pbg.tile([P, S * R1], bf16, tag="g")
                    for t in range(S):
                        ph = psh.tile([P, R1], f32, tag="ph")
                        nc.tensor.matmul(out=ph[:], lhsT=xa[:, t * P:(t + 1) * P],
                                         rhs=w1sa[:], start=True, stop=False)
                        nc.tensor.matmul(out=ph[:], lhsT=xb[:, t * P:(t + 1) * P],
                                         rhs=w1sb[:], start=False, stop=True)
                        if t % 2 == 0:
                            nc.scalar.copy(out=g[:, t * R1:(t + 1) * R1], in_=ph[:])
                        else:
                            nc.vector.tensor_copy(out=g[:, t * R1:(t + 1) * R1],
                                                  in_=ph[:])
                    # oT one-hots via PE broadcast of dlT + is_equal
                    dlTa = pbg.tile([1, S * P], bf16, tag="dlTa")
                    nc.sync.dma_start(out=dlTa[:],
                                      in_=dlT2_d[0:1, base * P:(base + S) * P])
                    oTA = pbo.tile([P, S * P], bf16, tag="oTA")
                    for st in range(0, S * P, 512):
                        w = min(512, S * P - st)
                        stp = psk.tile([P, 512], f32, tag="stp")
                        nc.tensor.matmul(out=stp[:, :w], lhsT=onk[:],
                                         rhs=dlTa[0:1, st:st + w],
                                         start=True, stop=True)
                        nc.vector.tensor_tensor(
                            out=oTA[:, st:st + w],
                            in0=iotc[:, 0:1].to_broadcast([P, w]),
                            in1=stp[:, :w],
                            op=mybir.AluOpType.is_equal)
                    if has_b:
                        dlTb = pbg.tile([1, S * P], bf16, tag="dlTb")
                        nc.sync.dma_start(out=dlTb[:],
                                          in_=dlT2_d[1:2, base * P:(base + S) * P])
                        oTB = pbo.tile([P, S * P], bf16, tag="oTB")
                        for st in range(0, S * P, 512):
                            w = min(512, S * P - st)
                            stp = psk.tile([P, 512], f32, tag="stp")
                            nc.tensor.matmul(out=stp[:, :w], lhsT=onk[:],
                                             rhs=dlTb[0:1, st:st + w],
                                             start=True, stop=True)
                            nc.vector.tensor_tensor(
                                out=oTB[:, st:st + w],
                                in0=iotc[:, 0:1].to_broadcast([P, w]),
                                in1=stp[:, :w],
                                op=mybir.AluOpType.is_equal)
                    # a_dst per slot (shares the stp bank ring)
                    pad = psk.tile([P, 512], f32, tag="stp")
                    for t in range(S):
                        bA = blo + int(sb["uA"][t])
                        bBl = int(sb["uB"][t])
                        nc.tensor.matmul(
                            out=pad[:, t * H:(t + 1) * H],
                            lhsT=oTA[:, t * P:(t + 1) * P],
                            rhs=adw[:, bA * H:(bA + 1) * H],
                            start=True, stop=(bBl < 0),
                            skip_group_check=True)
                        if bBl >= 0:
                            bB = blo + bBl
                            nc.tensor.matmul(
                                out=pad[:, t * H:(t + 1) * H],
                                lhsT=oTB[:, t * P:(t + 1) * P],
                                rhs=adw[:, bB * H:(bB + 1) * H],
                                start=False, stop=True,
                                skip_group_check=True)
                    # ex = exp(lrelu(asrc + adst))  [P, S*H] f32
                    ex = pbb.tile([P, S * H], f32, tag="ex")
                    nc.vector.tensor_tensor(
                        out=ex[:].rearrange("p (t h) -> p t h", t=S),
                        in0=_sub(g[:], HC, [[R1, S], [1, H]]),
                        in1=_sub(pad[:], 0, [[H, S], [1, H]]),
                        op=mybir.AluOpType.add)
                    tmp = pbb.tile([P, S * H], f32, tag="tmp")
                    nc.vector.tensor_scalar_mul(out=tmp[:], in0=ex[:], scalar1=NEG)
                    nc.vector.tensor_tensor(out=ex[:], in0=ex[:], in1=tmp[:],
                                            op=mybir.AluOpType.max)
                    nc.scalar.activation(out=ex[:], in_=ex[:], func=EXP)
                    # msg in place: h *= ex ; asrc cols := ex
                    nc.vector.tensor_tensor(
                        out=_sub(g[:], 0, [[R1, S], [C, H], [1, C]]),
                        in0=_sub(g[:], 0, [[R1, S], [C, H], [1, C]]),
                        in1=_sub(ex[:], 0, [[H, S], [1, H], [0, C]]),
                        op=mybir.AluOpType.mult)
                    nc.vector.tensor_copy(
                        out=_sub(g[:], HC, [[R1, S], [1, H]]),
                        in_=ex[:].rearrange("p (t h) -> p t h", t=S))
                    # oh one-hots  [P, S*P]
                    ohA = pbo.tile([P, S * P], bf16, tag="ohA")
                    nc.vector.tensor_tensor(
                        out=ohA[:].rearrange("p (t q) -> p t q", t=S),
                        in0=_sub(dlA[:], base, [[1, S], [0, P]]),
                        in1=_sub(iot[:], 0, [[0, S], [1, P]]),
                        op=mybir.AluOpType.is_equal)
                    if has_b:
                        ohB = pbo.tile([P, S * P], bf16, tag="ohB")
                        nc.vector.tensor_tensor(
                            out=ohB[:].rearrange("p (t q) -> p t q", t=S),
                            in0=_sub(dlB[:], base, [[1, S], [0, P]]),
                            in1=_sub(iot[:], 0, [[0, S], [1, P]]),
                            op=mybir.AluOpType.is_equal)
                    # scatter-add per block + epilogue
                    for bi in range(nblk):
                        b = blo + bi
                        t0, t1 = int(sb["t0"][bi]), int(sb["t1"][bi])
                        pso = psb.tile([P, R1], f32, tag="pso")
                        for t in range(t0, t1 + 1):
                            oh = ohA if int(sb["uA"][t]) == bi else ohB
                            nc.tensor.matmul(
                                out=pso[:],
                                lhsT=oh[:, t * P:(t + 1) * P],
                                rhs=g[:, t * R1:t * R1 + R1],
                                start=(t == t0), stop=(t == t1))
                        den = pbb.tile([P, H], f32, tag="den")
                        nc.vector.tensor_scalar_max(out=den[:],
                                                    in0=pso[:, HC:HC + H],
                                                    scalar1=1e-20)
                        rde = pbb.tile([P, H], f32, tag="rde")
                        nc.vector.reciprocal(out=rde[:], in_=den[:])
                        o1 = pbb.tile([P, HC], bf16, tag="o1")
                        for hh in range(H):
                            nc.vector.tensor_scalar_mul(
                                out=o1[:, hh * C:(hh + 1) * C],
                                in0=pso[:, hh * C:(hh + 1) * C],
                                scalar1=rde[:, hh:hh + 1])
                        nc.vector.tensor_tensor(out=o1[:], in0=o1[:], in1=b1s[:],
                                                op=mybir.AluOpType.add)
                        nc.vector.tensor_scalar_max(out=o1[:], in0=o1[:],
                                                    scalar1=0.0)
                        ph2 = ps2p.tile([P, 4], f32, tag="ph2")
                        for k in range(2):
                            ptr = pst.tile([P, P], bf16, tag="ptr")
                            nc.tensor.transpose(out=ptr[:],
                                                in_=o1[:, k * P:(k + 1) * P],
                                                identity=idn[:])
                            rT = pbb.tile([P, P], bf16, tag="rT")
                            nc.vector.tensor_copy(out=rT[:], in_=ptr[:])
                            nc.tensor.matmul(out=ph2[:], lhsT=rT[:],
                                             rhs=w2s[k][:],
                                             start=(k == 0), stop=(k == 1))
                        nc.vector.tensor_copy(out=h2c[:, b * 4:(b + 1) * 4],
                                              in_=ph2[:])
                        nc.vector.tensor_copy(out=ad2w[:, b:b + 1],
                                              in_=ph2[:, 3:4])

            # h2c -> h2loc DRAM  (full blocks + partial last)
            rows_last = NPC - (NB - 1) * P
            nc.sync.dma_start(
                out=bass.AP(h2loc, 0, [[4, P], [P * 4, NB - 1], [1, 4]]),
                in_=h2c[:, :(NB - 1) * 4])
            nc.sync.dma_start(
                out=bass.AP(h2loc, (NB - 1) * P * 4, [[4, rows_last], [1, 4]]),
                in_=h2c[:rows_last, (NB - 1) * 4:NB * 4])

            # ---------------- AllGather ------------------------------------
            nc.gpsimd.collective_compute(
                "AllGather", mybir.AluOpType.bypass,
                replica_groups=[list(range(NC))],
                ins=[h2loc[:, :]], outs=[h2tab[:, :]])
            h2tab16 = bass.AP(h2tab, 0, [[RL2, NT16], [1, RL2]])

            # ---------------- Phase C: L2 edge pass --------------------------
            with tc.tile_pool(name="pcg", bufs=2) as pcg, \
                 tc.tile_pool(name="pco", bufs=1) as pco, \
                 tc.tile_pool(name="pcb", bufs=2) as pcb, \
                 tc.tile_pool(name="psk2", bufs=1, space="PSUM") as psk2, \
                 tc.tile_pool(name="psc", bufs=4, space="PSUM") as psc:
                for sb in sbs:
                    base, S = sb["base"], sb["S"]
                    blo = sb["blo"]
                    nblk = sb["bhi"] - blo
                    has_b = bool((sb["uB"] >= 0).any())
                    g2 = pcg.tile([P, S * RL2], f32, tag="g2")
                    gather_split(g2, 0, S, RL2, h2tab16,
                                 ixs[:, base * 8:(base + S) * 8])
                    # lane extract: m2p[slot, 0:4] = h2tab16 row lane src%16
                    msk = pcb.tile([P, S * 16], bf16, tag="msk")
                    nc.vector.tensor_tensor(
                        out=msk[:].rearrange("p (t k) -> p t k", t=S),
                        in0=_sub(lsl[:], base, [[1, S], [0, 16]]),
                        in1=_sub(io16[:], 0, [[0, S], [1, 16]]),
                        op=mybir.AluOpType.is_equal)
                    tm2 = pcb.tile([P, S * RL2], f32, tag="tm2")
                    nc.vector.tensor_tensor(
                        out=_sub(tm2[:], 0, [[RL2, S], [16, 4], [1, 16]]),
                        in0=_sub(g2[:], 0, [[RL2, S], [1, 4], [4, 16]]),
                        in1=_sub(msk[:], 0, [[16, S], [0, 4], [1, 16]]),
                        op=mybir.AluOpType.mult)
                    m2p = pcb.tile([P, S * 4], f32, tag="m2p")
                    nc.vector.tensor_reduce(
                        out=m2p[:].rearrange("p (t c) -> p t c", t=S),
                        in_=_sub(tm2[:], 0, [[RL2, S], [16, 4], [1, 16]]),
                        axis=mybir.AxisListType.X,
                        op=mybir.AluOpType.add)
                    # oT one-hots (reuse same dlT data)
                    dlTa = pcg.tile([1, S * P], bf16, tag="dlTa2")
                    nc.sync.dma_start(out=dlTa[:],
                                      in_=dlT2_d[0:1, base * P:(base + S) * P])
                    oTA = pco.tile([P, S * P], bf16, tag="oTA2")
                    for st in range(0, S * P, 512):
                        w = min(512, S * P - st)
                        stp = psk2.tile([P, 512], f32, tag="stp2")
                        nc.tensor.matmul(out=stp[:, :w], lhsT=onk[:],
                                         rhs=dlTa[0:1, st:st + w],
                                         start=True, stop=True)
                        nc.vector.tensor_tensor(
                            out=oTA[:, st:st + w],
                            in0=iotc[:, 0:1].to_broadcast([P, w]),
                            in1=stp[:, :w],
                            op=mybir.AluOpType.is_equal)
                    if has_b:
                        dlTb = pcg.tile([1, S * P], bf16, tag="dlTb2")
                        nc.sync.dma_start(out=dlTb[:],
                                          in_=dlT2_d[1:2, base * P:(base + S) * P])
                        oTB = pco.tile([P, S * P], bf16, tag="oTB2")
                        for st in range(0, S * P, 512):
                            w = min(512, S * P - st)
                            stp = psk2.tile([P, 512], f32, tag="stp2")
                            nc.tensor.matmul(out=stp[:, :w], lhsT=onk[:],
                                             rhs=dlTb[0:1, st:st + w],
                                             start=True, stop=True)
                            nc.vector.tensor_tensor(
                                out=oTB[:, st:st + w],
                                in0=iotc[:, 0:1].to_broadcast([P, w]),
                                in1=stp[:, :w],
                                op=mybir.AluOpType.is_equal)
                    pad2 = psk2.tile([P, 512], f32, tag="stp2")
                    for t in range(S):
                        bA = blo + int(sb["uA"][t])
                        bBl = int(sb["uB"][t])
                        nc.tensor.matmul(
                            out=pad2[:, t:t + 1],
                            lhsT=oTA[:, t * P:(t + 1) * P],
                            rhs=ad2w[:, bA:bA + 1],
                            start=True, stop=(bBl < 0),
                            skip_group_check=True)
                        if bBl >= 0:
                            bB = blo + bBl
                            nc.tensor.matmul(
                                out=pad2[:, t:t + 1],
                                lhsT=oTB[:, t * P:(t + 1) * P],
                                rhs=ad2w[:, bB:bB + 1],
                                start=False, stop=True,
                                skip_group_check=True)
                    ex2 = pcb.tile([P, S], f32, tag="ex2")
                    nc.vector.tensor_tensor(
                        out=ex2[:],
                        in0=_sub(m2p[:], 2, [[4, S]]),
                        in1=_sub(pad2[:], 0, [[1, S]]),
                        op=mybir.AluOpType.add)
                    tm3 = pcb.tile([P, S], f32, tag="tm3")
                    nc.vector.tensor_scalar_mul(out=tm3[:], in0=ex2[:], scalar1=NEG)
                    nc.vector.tensor_tensor(out=ex2[:], in0=ex2[:], in1=tm3[:],
                                            op=mybir.AluOpType.max)
                    nc.scalar.activation(out=ex2[:], in_=ex2[:], func=EXP)
                    # m2 = [h2_0*ex | h2_1*ex | ex | ex]  bf16
                    m2 = pcb.tile([P, S * 4], bf16, tag="m2")
                    nc.vector.tensor_tensor(
                        out=_sub(m2[:], 0, [[4, S], [1, CLS]]),
                        in0=_sub(m2p[:], 0, [[4, S], [1, CLS]]),
                        in1=_sub(ex2[:], 0, [[1, S], [0, CLS]]),
                        op=mybir.AluOpType.mult)
                    nc.vector.tensor_copy(
                        out=_sub(m2[:], CLS, [[4, S], [1, 2]]),
                        in_=_sub(ex2[:], 0, [[1, S], [0, 2]]))
                    # oh one-hots
                    ohA = pco.tile([P, S * P], bf16, tag="ohA2")
                    nc.vector.tensor_tensor(
                        out=ohA[:].rearrange("p (t q) -> p t q", t=S),
                        in0=_sub(dlA[:], base, [[1, S], [0, P]]),
                        in1=_sub(iot[:], 0, [[0, S], [1, P]]),
                        op=mybir.AluOpType.is_equal)
                    if has_b:
                        ohB = pco.tile([P, S * P], bf16, tag="ohB2")
                        nc.vector.tensor_tensor(
                            out=ohB[:].rearrange("p (t q) -> p t q", t=S),
                            in0=_sub(dlB[:], base, [[1, S], [0, P]]),
                            in1=_sub(iot[:], 0, [[0, S], [1, P]]),
                            op=mybir.AluOpType.is_equal)
                    for bi in range(nblk):
                        b = blo + bi
                        t0, t1 = int(sb["t0"][bi]), int(sb["t1"][bi])
                        ps2 = psc.tile([P, 4], f32, tag="ps2")
                        for t in range(t0, t1 + 1):
                            oh = ohA if int(sb["uA"][t]) == bi else ohB
                            nc.tensor.matmul(
                                out=ps2[:],
                                lhsT=oh[:, t * P:(t + 1) * P],
                                rhs=m2[:, t * 4:(t + 1) * 4],
                                start=(t == t0), stop=(t == t1))
                        den2 = pcb.tile([P, 1], f32, tag="den2")
                        nc.vector.tensor_scalar_max(out=den2[:],
                                                    in0=ps2[:, 2:3],
                                                    scalar1=1e-20)
                        rd2 = pcb.tile([P, 1], f32, tag="rd2")
                        nc.vector.reciprocal(out=rd2[:], in_=den2[:])
                        v = pcb.tile([P, CLS], f32, tag="v")
                        nc.vector.tensor_scalar_mul(out=v[:], in0=ps2[:, 0:CLS],
                                                    scalar1=rd2[:, 0:1])
                        nc.vector.tensor_tensor(
                            out=vcol[:, b * CLS:(b + 1) * CLS],
                            in0=v[:], in1=b2s[:], op=mybir.AluOpType.add)

            # ---------------- batched log_softmax + output -------------------
            with tc.tile_pool(name="pf", bufs=1) as pf:
                mx = pf.tile([P, NB], f32)
                nc.vector.tensor_reduce(
                    out=mx[:], in_=vcol[:].rearrange("p (b c) -> p b c", b=NB),
                    axis=mybir.AxisListType.X, op=mybir.AluOpType.max)
                u = pf.tile([P, NB * CLS], f32)
                nc.vector.tensor_tensor(
                    out=u[:].rearrange("p (b c) -> p b c", b=NB),
                    in0=vcol[:].rearrange("p (b c) -> p b c", b=NB),
                    in1=_sub(mx[:], 0, [[1, NB], [0, CLS]]),
                    op=mybir.AluOpType.subtract)
                nc.scalar.activation(out=u[:], in_=u[:], func=EXP)
                sm = pf.tile([P, NB], f32)
                nc.vector.tensor_reduce(
                    out=sm[:], in_=u[:].rearrange("p (b c) -> p b c", b=NB),
                    axis=mybir.AxisListType.X, op=mybir.AluOpType.add)
                ls = pf.tile([P, NB], f32)
                nc.scalar.activation(out=ls[:], in_=sm[:], func=LN)
                nc.vector.tensor_tensor(out=ls[:], in0=ls[:], in1=mx[:],
                                        op=mybir.AluOpType.add)
                res = pf.tile([P, NB * CLS], f32)
                nc.vector.tensor_tensor(
                    out=res[:].rearrange("p (b c) -> p b c", b=NB),
                    in0=vcol[:].rearrange("p (b c) -> p b c", b=NB),
                    in1=_sub(ls[:], 0, [[1, NB], [0, CLS]]),
                    op=mybir.AluOpType.subtract)
                rows_last = NPC - (NB - 1) * P
                nc.sync.dma_start(
                    out=bass.AP(out_d, 0, [[CLS, P], [P * CLS, NB - 1], [1, CLS]]),
                    in_=res[:, :(NB - 1) * CLS])
                nc.sync.dma_start(
                    out=bass.AP(out_d, (NB - 1) * P * CLS,
                                [[CLS, rows_last], [1, CLS]]),
                    in_=res[:rows_last, (NB - 1) * CLS:NB * CLS])
    nc.finalize()
    return nc


def install_ntff_hook(so_path="/opt/axon/libaxon_pjrt.so"):
    import types
    import ctypes
    import contextlib
    import antenv

    if getattr(antenv, "axon_hooks", None) is not None:
        return
    lib = ctypes.CDLL(so_path)
    if not hasattr(lib, "axon_start_nrt_profile"):
        return
    lib.axon_start_nrt_profile.argtypes = [ctypes.POINTER(ctypes.c_int64),
                                           ctypes.c_size_t]
    lib.axon_start_nrt_profile.restype = ctypes.c_int64
    lib.axon_stop_nrt_profile.argtypes = [ctypes.c_char_p]
    lib.axon_stop_nrt_profile.restype = ctypes.c_int64

    @contextlib.contextmanager
    def _hook(output_dir, device_ids):
        import jax
        jax.devices()
        if device_ids:
            ids = (ctypes.c_int64 * len(device_ids))(*device_ids)
            rc = lib.axon_start_nrt_profile(ids, len(device_ids))
        else:
            rc = lib.axon_start_nrt_profile(None, 0)
        if rc != 0:
            raise RuntimeError(f"axon_start_nrt_profile rc={rc}")
        try:
            yield
        finally:
            n = lib.axon_stop_nrt_profile(str(output_dir).encode())
            print(f"ntff profile: {n} file(s) written to {output_dir}")

    mod = types.ModuleType("antenv.axon_hooks")
    _reg = [_hook]
    mod.set_axon_ntff_profile_hook = lambda h: _reg.__setitem__(0, h)
    mod.get_axon_ntff_profile_hook = lambda: _reg[0]
    sys.modules["antenv.axon_hooks"] = mod
    antenv.axon_hooks = mod


def run(inputs, cfg, trace=False, **kwargs):
    if trace:
        install_ntff_hook()
    in_maps, meta = prep(inputs, cfg)
    nc = build(meta)
    res = bass_utils.run_bass_kernel_spmd(
        nc, in_maps, core_ids=list(range(cfg["NC"])), trace=trace, **kwargs)
    out = np.concatenate([res.results[c]["out"] for c in range(cfg["NC"])], axis=0)
    return out, res


# ----------------------------------------------------------------------------
# harness entry point
# ----------------------------------------------------------------------------

_CFG = dict(N=100000, F=165, H=4, C=64, CLS=2, NC=8)


def kernel(**inputs):
    """Full (unsharded) inputs -> full [N, 2] float32 log-softmax output.

    Shards edges by destination-node range across the 8 NeuronCores,
    compiles and runs the Bass/Tile kernel via run_bass_kernel_spmd,
    and concatenates the per-core output slices.
    """
    out, _ = run(inputs, _CFG, trace=False)
    return np.ascontiguousarray(out.astype(np.float32))


# revision 25
# speedup vs baseline: 3.0960x; 1.5286x over previous
"""GAT 2-layer message-passing network on 8 TRN2 NeuronCores (Bass/Tile).

v3: gather-free L1 via host-side edge-slot materialization of x.

Strategy (dst-sharded, dense slot layout):
 - Host: add self loops, sort edges by dst, shard dst-node ranges across
   cores (core c owns nodes [c*NPC, (c+1)*NPC) and ALL edges into them).
 - Slots: per superblock of SBG=3 dst-blocks, each core's edges packed
   DENSELY into Tsb = ceil(max_core_count/128) tiles (common layout across
   cores; only trailing padding). A tile touches at most 2 dst blocks
   (maxU=2): host assigns each slot to one-hot set A or B.
 - L1 (Phase B): host pre-gathers x rows into slot order (xgaT [165, SLOTS]
   bf16). Device: h_slot = xga @ [W1|Wsrc] per tile (PE), a_dst via
   oT-matmul against a local per-block table, ex = exp(lrelu(asrc+adst)),
   msg = [h*ex | ex], scatter-add per block via one-hot matmuls, normalize,
   +b1, relu, then h2 = o1 @ [W2|Wsrc2|Wdst2] -> h2c [P, NB*4].
 - AllGather h2loc [NPC,4] f32 -> h2tab [N,4] f32 (= gather table viewed as
   [N/16, 64]: 16 nodes per 256B row; NO repack needed).
 - L2 (Phase C): same slots: dma_gather h2tab16 rows by src//16 (single
   chunk, int16), on-chip lane extract by src%16, ex2 via oT2/adst2 table,
   4-wide messages, one-hot scatter, normalize; batched log_softmax at end.
"""
import sys

if "/opt/trn_rl_repo" not in sys.path:
    sys.path.insert(0, "/opt/trn_rl_repo")

import math
import numpy as np
import ml_dtypes

import concourse.bass as bass
import concourse.bacc as bacc
import concourse.mybir as mybir
import concourse.tile as tile
from concourse import bass_utils

P = 128
NEG = 0.2
NQUEUE = 4
SBG = 3
MAXT = 7

# Tile's DMASW sem-lane assignment round-robins over all Pool DMAs, which
# breaks the per-lane FIFO assumption when SWDGE DMAs run on multiple queues
# (out-of-order completion across queues under one counting sem). Patch the
# lane choice to lane == queue_num: per-lane FIFO again holds (each HW ring
# drains in order), and queues get independent lanes.
from concourse import tile_sem_assignment as _tsa  # noqa: E402

if not getattr(_tsa.TileClockTick, "_qaware_patched", False):
    _orig_assign_tick = _tsa.TileClockTick._assign_tick

    def _qaware_assign_tick(self, inst):
        if (isinstance(inst, _tsa.DMAInst)
                and inst.engine == mybir.EngineType.Pool):
            self.next_sw_dma_idx = getattr(inst, "queue_num", 0) or 0
        return _orig_assign_tick(self, inst)

    _tsa.TileClockTick._assign_tick = _qaware_assign_tick
    _tsa.TileClockTick._qaware_patched = True


def _wrap16(flat):
    """[n] -> [128, n//16] wrapped in 16 partitions, replicated x8."""
    w = flat.reshape(-1, 16).T
    return np.tile(w, (8, 1))


# ----------------------------------------------------------------------------
# host-side data prep
# ----------------------------------------------------------------------------

def prep(inputs, cfg):
    N, F, H, C, CLS, NC = cfg["N"], cfg["F"], cfg["H"], cfg["C"], cfg["CLS"], cfg["NC"]
    x = np.asarray(inputs["x"], np.float32)
    ei = np.asarray(inputs["edge_index"])
    W1 = np.asarray(inputs["W1"], np.float32)
    as1 = np.asarray(inputs["att_src1"], np.float32)
    ad1 = np.asarray(inputs["att_dst1"], np.float32)
    b1 = np.asarray(inputs["b1"], np.float32)
    W2 = np.asarray(inputs["W2"], np.float32)
    as2 = np.asarray(inputs["att_src2"], np.float32)
    ad2 = np.asarray(inputs["att_dst2"], np.float32)
    b2 = np.asarray(inputs["b2"], np.float32)

    HC = H * C                          # 256
    R1 = HC + H                         # 260 = [h | asrc]
    NPC = N // NC
    NB = math.ceil(NPC / P)
    NPCp = NB * P
    bf16 = ml_dtypes.bfloat16

    # ---- weights / constants -------------------------------------------------
    W1r = W1.reshape(F, H, C)
    Wsrc = np.einsum("fhc,hc->fh", W1r, as1)
    Wdst = np.einsum("fhc,hc->fh", W1r, ad1)
    W1s = np.concatenate([W1, Wsrc], axis=1).astype(bf16)     # [F, R1]
    Wd = Wdst.astype(bf16)                                    # [F, H]
    Wsrc2 = W2 @ as2.reshape(CLS, 1)
    Wdst2 = W2 @ ad2.reshape(CLS, 1)
    W2aug = np.concatenate([W2, Wsrc2, Wdst2], axis=1).astype(bf16)  # [HC, 4]

    b1rep = np.tile(b1[None, :], (P, 1)).astype(bf16)
    b2rep = np.tile(b2[None, :], (P, 1)).astype(np.float32)
    ident = np.eye(P, dtype=bf16)

    # ---- edges ---------------------------------------------------------------
    src_all = np.concatenate([ei[0], np.arange(N, dtype=ei.dtype)]).astype(np.int64)
    dst_all = np.concatenate([ei[1], np.arange(N, dtype=ei.dtype)]).astype(np.int64)
    order = np.argsort(dst_all, kind="stable")
    src_s = src_all[order]
    dst_s = dst_all[order]

    # per (core, block) counts
    cnt = np.zeros((NC, NB), np.int64)
    for c in range(NC):
        for b in range(NB):
            base = c * NPC + b * P
            hi = min(base + P, (c + 1) * NPC)
            cnt[c, b] = (np.searchsorted(dst_s, hi) -
                         np.searchsorted(dst_s, base))

    # superblock metadata (common across cores)
    sbs = []
    tile_base = 0
    nsb = math.ceil(NB / SBG)
    for s in range(nsb):
        blo, bhi = s * SBG, min((s + 1) * SBG, NB)
        persb = cnt[:, blo:bhi]
        Tsb = int(np.ceil(persb.sum(axis=1).max() / P))
        t0 = np.zeros(bhi - blo, np.int64)
        t1 = np.zeros(bhi - blo, np.int64)
        for bi in range(bhi - blo):
            lo = persb[:, :bi].sum(axis=1)
            hi2 = persb[:, :bi + 1].sum(axis=1)
            t0[bi] = lo.min() // P
            t1[bi] = min(int(np.ceil(hi2 / P).max()) - 1, Tsb - 1)
        # per tile: ordered union of touched blocks (local block index)
        uA = np.full(Tsb, -1, np.int64)
        uB = np.full(Tsb, -1, np.int64)
        for t in range(Tsb):
            u = [bi for bi in range(bhi - blo) if t0[bi] <= t <= t1[bi]]
            assert 1 <= len(u) <= 2, (s, t, u)
            uA[t] = u[0]
            if len(u) == 2:
                uB[t] = u[1]
        sbs.append(dict(base=tile_base, S=Tsb, blo=blo, bhi=bhi,
                        t0=t0, t1=t1, uA=uA, uB=uB))
        tile_base += Tsb
    Tsum = tile_base
    SLOT = Tsum * P
    S_MAX = max(sb["S"] for sb in sbs)

    # ---- per-core slot arrays ------------------------------------------------
    x_bf = x.astype(bf16)
    in_maps = []
    shared = {
        "w1s_a": np.ascontiguousarray(W1s[:P]),
        "w1s_b": np.ascontiguousarray(W1s[P:]),
        "wd_a": np.ascontiguousarray(Wd[:P]),
        "wd_b": np.ascontiguousarray(Wd[P:]),
        "w2aug": W2aug, "b1rep": b1rep, "b2rep": b2rep, "ident": ident,
    }
    iotaP = np.arange(P)
    for c in range(NC):
        srcslot = np.zeros(SLOT, np.int64)
        dlocA = np.full(SLOT, 255, np.int64)
        dlocB = np.full(SLOT, 255, np.int64)
        core_lo = np.searchsorted(dst_s, c * NPC)
        core_hi = np.searchsorted(dst_s, (c + 1) * NPC)
        cs = src_s[core_lo:core_hi]
        cd = dst_s[core_lo:core_hi]
        for sb in sbs:
            base_n = c * NPC + sb["blo"] * P
            hi_n = min(c * NPC + sb["bhi"] * P, (c + 1) * NPC)
            lo_i = np.searchsorted(cd, base_n)
            hi_i = np.searchsorted(cd, hi_n)
            es = cs[lo_i:hi_i]
            ed = cd[lo_i:hi_i]
            n = len(es)
            assert n <= sb["S"] * P
            s0 = sb["base"] * P
            srcslot[s0:s0 + n] = es
            # block-local dst and A/B role per slot
            pos = np.arange(n)
            trel = pos // P
            bloc = (ed - c * NPC) // P - sb["blo"]          # local block idx
            dv = ed - c * NPC - (bloc + sb["blo"]) * P
            isA = sb["uA"][trel] == bloc
            isB = sb["uB"][trel] == bloc
            assert np.all(isA | isB), (c, sb["base"])
            dlocA[s0 + pos[isA]] = dv[isA]
            dlocB[s0 + pos[isB]] = dv[isB]
        m = dict(shared)
        xga = np.ascontiguousarray(x_bf[srcslot].T)         # [F, SLOT]
        m["xga_a"] = np.ascontiguousarray(xga[:P])
        m["xga_b"] = np.ascontiguousarray(xga[P:])
        xl = np.zeros((F, NPCp), dtype=bf16)
        xl[:, :NPC] = x_bf[c * NPC:(c + 1) * NPC].T
        m["xtl_a"] = np.ascontiguousarray(xl[:P])
        m["xtl_b"] = np.ascontiguousarray(xl[P:])
        # h2 gather idx: table = [NC*P, NB*4] f32 (partition-major per core)
        sc = srcslot // NPC
        loc = srcslot % NPC
        flat = sc * (P * NB) + (loc % P) * NB + loc // P    # row-of-4 index
        m["ihsrc16"] = _wrap16((flat // 16).astype(np.int16))
        lane = flat % 16
        # host-prebuilt one-hots and lane masks
        dA2 = dlocA.reshape(Tsum, P)                        # [t, s]
        dB2 = dlocB.reshape(Tsum, P)
        ohA = (dA2[:, :, None] == iotaP[None, None, :])     # [t, s, d]
        ohB = (dB2[:, :, None] == iotaP[None, None, :])
        m["ohA"] = ohA.transpose(1, 0, 2).reshape(P, Tsum * P).astype(bf16)
        m["ohB"] = ohB.transpose(1, 0, 2).reshape(P, Tsum * P).astype(bf16)
        m["oTA"] = ohA.transpose(2, 0, 1).reshape(P, Tsum * P).astype(bf16)
        m["oTB"] = ohB.transpose(2, 0, 1).reshape(P, Tsum * P).astype(bf16)
        msk = (lane.reshape(Tsum, P)[:, :, None] ==
               np.arange(16)[None, None, :])                # [t, s, 16]
        m["msk"] = msk.transpose(1, 0, 2).reshape(P, Tsum * 16).astype(bf16)
        in_maps.append(m)

    meta = dict(cfg, R1=R1, HC=HC, NPC=NPC, NPCp=NPCp, NB=NB, Tsum=Tsum,
                SLOT=SLOT, S_MAX=S_MAX, sbs=sbs)
    return in_maps, meta


# ----------------------------------------------------------------------------
# device program
# ----------------------------------------------------------------------------

def _sub(ap, elem_off, dims):
    return bass.AP(ap.tensor, ap.offset + elem_off, [ap.ap[0], *list(dims)])


def build(meta, nc=None):
    N, F, H, C, CLS = meta["N"], meta["F"], meta["H"], meta["C"], meta["CLS"]
    NC, R1, HC = meta["NC"], meta["R1"], meta["HC"]
    NPC, NPCp, NB = meta["NPC"], meta["NPCp"], meta["NB"]
    Tsum, SLOT = meta["Tsum"], meta["SLOT"]
    sbs = meta["sbs"]
    FB = F - P                           # 37
    NT16 = N // 16                       # h2tab rows of 16 nodes
    RL2 = 64                             # f32 elems per 256B gather row

    f32, bf16, i16 = mybir.dt.float32, mybir.dt.bfloat16, mybir.dt.int16
    EXP = mybir.ActivationFunctionType.Exp
    LN = mybir.ActivationFunctionType.Ln

    if nc is None:
        nc = bacc.Bacc("TRN2", target_bir_lowering=False, debug=False,
                       num_devices=NC, num_swdge_queues=NQUEUE)

    qrr = [0]

    def gather_split(out_tile, rel, segT, elem, table, ix_tile):
        done = 0
        while done < segT:
            tt = min(MAXT, segT - done)
            r = rel + done
            nc.gpsimd.dma_gather(
                bass.AP(out_tile[:].tensor, out_tile[:].offset + r * elem,
                        [out_tile[:].ap[0], [elem, tt], [1, elem]]),
                table,
                ix_tile[:, r * 8:(r + tt) * 8],
                tt * P, tt * P, elem,
                queue_num=qrr[0] % NQUEUE,
            )
            qrr[0] += 1
            done += tt

    xga_a_d = nc.dram_tensor("xga_a", [P, SLOT], bf16, kind="ExternalInput")
    xga_b_d = nc.dram_tensor("xga_b", [FB, SLOT], bf16, kind="ExternalInput")
    xtl_a_d = nc.dram_tensor("xtl_a", [P, NPCp], bf16, kind="ExternalInput")
    xtl_b_d = nc.dram_tensor("xtl_b", [FB, NPCp], bf16, kind="ExternalInput")
    w1s_a_d = nc.dram_tensor("w1s_a", [P, R1], bf16, kind="ExternalInput")
    w1s_b_d = nc.dram_tensor("w1s_b", [FB, R1], bf16, kind="ExternalInput")
    wd_a_d = nc.dram_tensor("wd_a", [P, H], bf16, kind="ExternalInput")
    wd_b_d = nc.dram_tensor("wd_b", [FB, H], bf16, kind="ExternalInput")
    w2aug_d = nc.dram_tensor("w2aug", [HC, 4], bf16, kind="ExternalInput")
    b1rep_d = nc.dram_tensor("b1rep", [P, HC], bf16, kind="ExternalInput")
    b2rep_d = nc.dram_tensor("b2rep", [P, CLS], f32, kind="ExternalInput")
    ident_d = nc.dram_tensor("ident", [P, P], bf16, kind="ExternalInput")
    ihsrc_d = nc.dram_tensor("ihsrc16", [P, Tsum * 8], i16, kind="ExternalInput")
    ohA_d = nc.dram_tensor("ohA", [P, SLOT], bf16, kind="ExternalInput")
    ohB_d = nc.dram_tensor("ohB", [P, SLOT], bf16, kind="ExternalInput")
    oTA_d = nc.dram_tensor("oTA", [P, SLOT], bf16, kind="ExternalInput")
    oTB_d = nc.dram_tensor("oTB", [P, SLOT], bf16, kind="ExternalInput")
    msk_d = nc.dram_tensor("msk", [P, Tsum * 16], bf16, kind="ExternalInput")
    out_d = nc.dram_tensor("out", [P, NB * CLS], f32, kind="ExternalOutput")

    h2locp = nc.dram_tensor("h2locp", [P, NB * 4], f32, kind="Internal")
    h2tabp = nc.dram_tensor("h2tabp", [NC * P, NB * 4], f32, kind="Internal",
                            addr_space="Shared" if NC > 4 else "Local")

    with tile.TileContext(nc) as tc:
        with tc.tile_pool(name="const", bufs=1) as cp:
            w1sa = cp.tile([P, R1], bf16)
            nc.sync.dma_start(out=w1sa[:], in_=w1s_a_d[:, :])
            w1sb = cp.tile([FB, R1], bf16)
            nc.sync.dma_start(out=w1sb[:], in_=w1s_b_d[:, :])
            w2s = []
            for k in range(2):
                w2k = cp.tile([P, 4], bf16, name=f"w2k{k}")
                nc.sync.dma_start(out=w2k[:], in_=w2aug_d[k * P:(k + 1) * P, :])
                w2s.append(w2k)
            b1s = cp.tile([P, HC], bf16)
            nc.sync.dma_start(out=b1s[:], in_=b1rep_d[:, :])
            b2s = cp.tile([P, CLS], f32)
            nc.sync.dma_start(out=b2s[:], in_=b2rep_d[:, :])
            idn = cp.tile([P, P], bf16)
            nc.sync.dma_start(out=idn[:], in_=ident_d[:, :])
            ixs = cp.tile([P, Tsum * 8], i16)
            nc.sync.dma_start(out=ixs[:], in_=ihsrc_d[:, :])
            adw = cp.tile([P, NB * H], bf16)      # a_dst1 per local node
            ad2w = cp.tile([P, NB], bf16)         # a_dst2 per local node
            h2c = cp.tile([P, NB * 4], f32)       # h2 collect [out2|asrc2|adst2]
            vcol = cp.tile([P, NB * CLS], f32)    # L2 logits collect

            # ---------------- Phase A-mini: local a_dst1 table ---------------
            with tc.tile_pool(name="pa", bufs=1) as pa, \
                 tc.tile_pool(name="psa", bufs=4, space="PSUM") as psa:
                xla = pa.tile([P, NPCp], bf16)
                nc.sync.dma_start(out=xla[:], in_=xtl_a_d[:, :])
                xlb = pa.tile([FB, NPCp], bf16)
                nc.sync.dma_start(out=xlb[:], in_=xtl_b_d[:, :])
                wda = pa.tile([P, H], bf16)
                nc.sync.dma_start(out=wda[:], in_=wd_a_d[:, :])
                wdb = pa.tile([FB, H], bf16)
                nc.sync.dma_start(out=wdb[:], in_=wd_b_d[:, :])
                for nt in range(NB):
                    ps = psa.tile([P, H], f32, tag="ps")
                    nc.tensor.matmul(out=ps[:], lhsT=xla[:, nt * P:(nt + 1) * P],
                                     rhs=wda[:], start=True, stop=False)
                    nc.tensor.matmul(out=ps[:], lhsT=xlb[:, nt * P:(nt + 1) * P],
                                     rhs=wdb[:], start=False, stop=True)
                    nc.vector.tensor_copy(out=adw[:, nt * H:(nt + 1) * H],
                                          in_=ps[:])

            # ---------------- Phase B: L1 edge pass --------------------------
            with tc.tile_pool(name="pbg", bufs=2) as pbg, \
                 tc.tile_pool(name="pbo", bufs=2) as pbo, \
                 tc.tile_pool(name="pbb", bufs=2) as pbb, \
                 tc.tile_pool(name="psh", bufs=2, space="PSUM") as psh, \
                 tc.tile_pool(name="psk", bufs=1, space="PSUM") as psk, \
                 tc.tile_pool(name="psb", bufs=3, space="PSUM") as psb, \
                 tc.tile_pool(name="pst", bufs=1, space="PSUM") as pst, \
                 tc.tile_pool(name="ps2", bufs=1, space="PSUM") as ps2p:
                for sb in sbs:
                    base, S = sb["base"], sb["S"]
                    blo = sb["blo"]
                    nblk = sb["bhi"] - blo
                    has_b = bool((sb["uB"] >= 0).any())
                    xa = pbg.tile([P, S * P], bf16, tag="xa")
                    nc.sync.dma_start(out=xa[:],
                                      in_=xga_a_d[:, base * P:(base + S) * P])
                    xb = pbg.tile([FB, S * P], bf16, tag="xb")
                    nc.sync.dma_start(out=xb[:],
                                      in_=xga_b_d[:, base * P:(base + S) * P])
                    # one-hots (host-prebuilt)
                    oTA = pbo.tile([P, S * P], bf16, tag="oTA")
                    nc.sync.dma_start(out=oTA[:],
                                      in_=oTA_d[:, base * P:(base + S) * P])
                    ohA = pbo.tile([P, S * P], bf16, tag="ohA")
                    nc.sync.dma_start(out=ohA[:],
                                      in_=ohA_d[:, base * P:(base + S) * P])
                    if has_b:
                        oTB = pbo.tile([P, S * P], bf16, tag="oTB")
                        nc.sync.dma_start(out=oTB[:],
                                          in_=oTB_d[:, base * P:(base + S) * P])
                        ohB = pbo.tile([P, S * P], bf16, tag="ohB")
                        nc.sync.dma_start(out=ohB[:],
                                          in_=ohB_d[:, base * P:(base + S) * P])
                    # h per slot -> psum -> g (bf16)
                    g = pbg.tile([P, S * R1], bf16, tag="g")
                    for t in range(S):
                        ph = psh.tile([P, R1], f32, tag="ph")
                        nc.tensor.matmul(out=ph[:], lhsT=xa[:, t * P:(t + 1) * P],
                                         rhs=w1sa[:], start=True, stop=False)
                        nc.tensor.matmul(out=ph[:], lhsT=xb[:, t * P:(t + 1) * P],
                                         rhs=w1sb[:], start=False, stop=True)
                        if t % 2 == 0:
                            nc.scalar.copy(out=g[:, t * R1:(t + 1) * R1], in_=ph[:])
                        else:
                            nc.vector.tensor_copy(out=g[:, t * R1:(t + 1) * R1],
                                                  in_=ph[:])
                    # a_dst per slot
                    pad = psk.tile([P, 512], f32, tag="stp")
                    for t in range(S):
                        bA = blo + int(sb["uA"][t])
                        bBl = int(sb["uB"][t])
                        nc.tensor.matmul(
                            out=pad[:, t * H:(t + 1) * H],
                            lhsT=oTA[:, t * P:(t + 1) * P],
                            rhs=adw[:, bA * H:(bA + 1) * H],
                            start=True, stop=(bBl < 0),
                            skip_group_check=True)
                        if bBl >= 0:
                            bB = blo + bBl
                            nc.tensor.matmul(
                                out=pad[:, t * H:(t + 1) * H],
                                lhsT=oTB[:, t * P:(t + 1) * P],
                                rhs=adw[:, bB * H:(bB + 1) * H],
                                start=False, stop=True,
                                skip_group_check=True)
                    # ex = exp(lrelu(asrc + adst))  [P, S*H] f32
                    ex = pbb.tile([P, S * H], f32, tag="ex")
                    nc.vector.tensor_tensor(
                        out=ex[:].rearrange("p (t h) -> p t h", t=S),
                        in0=_sub(g[:], HC, [[R1, S], [1, H]]),
                        in1=_sub(pad[:], 0, [[H, S], [1, H]]),
                        op=mybir.AluOpType.add)
                    tmp = pbb.tile([P, S * H], f32, tag="tmp")
                    nc.vector.tensor_scalar_mul(out=tmp[:], in0=ex[:], scalar1=NEG)
                    nc.vector.tensor_tensor(out=ex[:], in0=ex[:], in1=tmp[:],
                                            op=mybir.AluOpType.max)
                    nc.scalar.activation(out=ex[:], in_=ex[:], func=EXP)
                    # msg in place: h *= ex ; asrc cols := ex
                    nc.vector.tensor_tensor(
                        out=_sub(g[:], 0, [[R1, S], [C, H], [1, C]]),
                        in0=_sub(g[:], 0, [[R1, S], [C, H], [1, C]]),
                        in1=_sub(ex[:], 0, [[H, S], [1, H], [0, C]]),
                        op=mybir.AluOpType.mult)
                    nc.vector.tensor_copy(
                        out=_sub(g[:], HC, [[R1, S], [1, H]]),
                        in_=ex[:].rearrange("p (t h) -> p t h", t=S))
                    # scatter-add per block + epilogue
                    for bi in range(nblk):
                        b = blo + bi
                        t0, t1 = int(sb["t0"][bi]), int(sb["t1"][bi])
                        pso = psb.tile([P, R1], f32, tag="pso")
                        for t in range(t0, t1 + 1):
                            oh = ohA if int(sb["uA"][t]) == bi else ohB
                            nc.tensor.matmul(
                                out=pso[:],
                                lhsT=oh[:, t * P:(t + 1) * P],
                                rhs=g[:, t * R1:t * R1 + R1],
                                start=(t == t0), stop=(t == t1))
                        den = pbb.tile([P, H], f32, tag="den")
                        nc.vector.tensor_scalar_max(out=den[:],
                                                    in0=pso[:, HC:HC + H],
                                                    scalar1=1e-20)
                        rde = pbb.tile([P, H], f32, tag="rde")
                        nc.vector.reciprocal(out=rde[:], in_=den[:])
                        o1 = pbb.tile([P, HC], bf16, tag="o1")
                        for hh in range(H):
                            nc.vector.tensor_scalar_mul(
                                out=o1[:, hh * C:(hh + 1) * C],
                                in0=pso[:, hh * C:(hh + 1) * C],
                                scalar1=rde[:, hh:hh + 1])
                        nc.vector.tensor_tensor(out=o1[:], in0=o1[:], in1=b1s[:],
                                                op=mybir.AluOpType.add)
                        nc.vector.tensor_scalar_max(out=o1[:], in0=o1[:],
                                                    scalar1=0.0)
                        ph2 = ps2p.tile([P, 4], f32, tag="ph2")
                        for k in range(2):
                            ptr = pst.tile([P, P], bf16, tag="ptr")
                            nc.tensor.transpose(out=ptr[:],
                                                in_=o1[:, k * P:(k + 1) * P],
                                                identity=idn[:])
                            rT = pbb.tile([P, P], bf16, tag="rT")
                            nc.vector.tensor_copy(out=rT[:], in_=ptr[:])
                            nc.tensor.matmul(out=ph2[:], lhsT=rT[:],
                                             rhs=w2s[k][:],
                                             start=(k == 0), stop=(k == 1))
                        nc.vector.tensor_copy(out=h2c[:, b * 4:(b + 1) * 4],
                                              in_=ph2[:])
                        nc.vector.tensor_copy(out=ad2w[:, b:b + 1],
                                              in_=ph2[:, 3:4])

            # h2c -> h2locp DRAM (partition-major, one dense DMA)
            nc.sync.dma_start(out=h2locp[:, :], in_=h2c[:])

            # ---------------- AllGather ------------------------------------
            nc.gpsimd.collective_compute(
                "AllGather", mybir.AluOpType.bypass,
                replica_groups=[list(range(NC))],
                ins=[h2locp[:, :]], outs=[h2tabp[:, :]])
            NROW16 = NC * P * NB * 4 // RL2
            h2tab16 = bass.AP(h2tabp, 0, [[RL2, NROW16], [1, RL2]])

            # ---------------- Phase C: L2 edge pass --------------------------
            with tc.tile_pool(name="pcg", bufs=2) as pcg, \
                 tc.tile_pool(name="pco", bufs=1) as pco, \
                 tc.tile_pool(name="pcb", bufs=2) as pcb, \
                 tc.tile_pool(name="psk2", bufs=1, space="PSUM") as psk2, \
                 tc.tile_pool(name="psc", bufs=4, space="PSUM") as psc:
                for sb in sbs:
                    base, S = sb["base"], sb["S"]
                    blo = sb["blo"]
                    nblk = sb["bhi"] - blo
                    has_b = bool((sb["uB"] >= 0).any())
                    g2 = pcg.tile([P, S * RL2], f32, tag="g2")
                    gather_split(g2, 0, S, RL2, h2tab16,
                                 ixs[:, base * 8:(base + S) * 8])
                    # one-hots + lane masks (host-prebuilt)
                    oTA = pco.tile([P, S * P], bf16, tag="oTA2")
                    nc.sync.dma_start(out=oTA[:],
                                      in_=oTA_d[:, base * P:(base + S) * P])
                    ohA = pco.tile([P, S * P], bf16, tag="ohA2")
                    nc.sync.dma_start(out=ohA[:],
                                      in_=ohA_d[:, base * P:(base + S) * P])
                    if has_b:
                        oTB = pco.tile([P, S * P], bf16, tag="oTB2")
                        nc.sync.dma_start(out=oTB[:],
                                          in_=oTB_d[:, base * P:(base + S) * P])
                        ohB = pco.tile([P, S * P], bf16, tag="ohB2")
                        nc.sync.dma_start(out=ohB[:],
                                          in_=ohB_d[:, base * P:(base + S) * P])
                    msk = pcb.tile([P, S * 16], bf16, tag="msk")
                    nc.sync.dma_start(out=msk[:],
                                      in_=msk_d[:, base * 16:(base + S) * 16])
                    # lane extract: m2p[slot, 0:4] = h2tab16 row lane
                    tm2 = pcb.tile([P, S * RL2], f32, tag="tm2")
                    nc.vector.tensor_tensor(
                        out=_sub(tm2[:], 0, [[RL2, S], [16, 4], [1, 16]]),
                        in0=_sub(g2[:], 0, [[RL2, S], [1, 4], [4, 16]]),
                        in1=_sub(msk[:], 0, [[16, S], [0, 4], [1, 16]]),
                        op=mybir.AluOpType.mult)
                    m2p = pcb.tile([P, S * 4], f32, tag="m2p")
                    nc.vector.tensor_reduce(
                        out=m2p[:].rearrange("p (t c) -> p t c", t=S),
                        in_=_sub(tm2[:], 0, [[RL2, S], [16, 4], [1, 16]]),
                        axis=mybir.AxisListType.X,
                        op=mybir.AluOpType.add)
                    pad2 = psk2.tile([P, 512], f32, tag="stp2")
                    for t in range(S):
                        bA = blo + int(sb["uA"][t])
                        bBl = int(sb["uB"][t])
                        nc.tensor.matmul(
                            out=pad2[:, t:t + 1],
                            lhsT=oTA[:, t * P:(t + 1) * P],
                            rhs=ad2w[:, bA:bA + 1],
                            start=True, stop=(bBl < 0),
                            skip_group_check=True)
                        if bBl >= 0:
                            bB = blo + bBl
                            nc.tensor.matmul(
                                out=pad2[:, t:t + 1],
                                lhsT=oTB[:, t * P:(t + 1) * P],
                                rhs=ad2w[:, bB:bB + 1],
                                start=False, stop=True,
                                skip_group_check=True)
                    ex2 = pcb.tile([P, S], f32, tag="ex2")
                    nc.vector.tensor_tensor(
                        out=ex2[:],
                        in0=_sub(m2p[:], 2, [[4, S]]),
                        in1=_sub(pad2[:], 0, [[1, S]]),
                        op=mybir.AluOpType.add)
                    tm3 = pcb.tile([P, S], f32, tag="tm3")
                    nc.vector.tensor_scalar_mul(out=tm3[:], in0=ex2[:], scalar1=NEG)
                    nc.vector.tensor_tensor(out=ex2[:], in0=ex2[:], in1=tm3[:],
                                            op=mybir.AluOpType.max)
                    nc.scalar.activation(out=ex2[:], in_=ex2[:], func=EXP)
                    # m2 = [h2_0*ex | h2_1*ex | ex | ex]  bf16
                    m2 = pcb.tile([P, S * 4], bf16, tag="m2")
                    nc.vector.tensor_tensor(
                        out=_sub(m2[:], 0, [[4, S], [1, CLS]]),
                        in0=_sub(m2p[:], 0, [[4, S], [1, CLS]]),
                        in1=_sub(ex2[:], 0, [[1, S], [0, CLS]]),
                        op=mybir.AluOpType.mult)
                    nc.vector.tensor_copy(
                        out=_sub(m2[:], CLS, [[4, S], [1, 2]]),
                        in_=_sub(ex2[:], 0, [[1, S], [0, 2]]))
                    for bi in range(nblk):
                        b = blo + bi
                        t0, t1 = int(sb["t0"][bi]), int(sb["t1"][bi])
                        ps2 = psc.tile([P, 4], f32, tag="ps2")
                        for t in range(t0, t1 + 1):
                            oh = ohA if int(sb["uA"][t]) == bi else ohB
                            nc.tensor.matmul(
                                out=ps2[:],
                                lhsT=oh[:, t * P:(t + 1) * P],
                                rhs=m2[:, t * 4:(t + 1) * 4],
                                start=(t == t0), stop=(t == t1))
                        den2 = pcb.tile([P, 1], f32, tag="den2")
                        nc.vector.tensor_scalar_max(out=den2[:],
                                                    in0=ps2[:, 2:3],
                                                    scalar1=1e-20)
                        rd2 = pcb.tile([P, 1], f32, tag="rd2")
                        nc.vector.reciprocal(out=rd2[:], in_=den2[:])
                        v = pcb.tile([P, CLS], f32, tag="v")
                        nc.vector.tensor_scalar_mul(out=v[:], in0=ps2[:, 0:CLS],
                                                    scalar1=rd2[:, 0:1])
                        nc.vector.tensor_tensor(
                            out=vcol[:, b * CLS:(b + 1) * CLS],
                            in0=v[:], in1=b2s[:], op=mybir.AluOpType.add)

            # ---------------- batched log_softmax + output -------------------
            with tc.tile_pool(name="pf", bufs=1) as pf:
                mx = pf.tile([P, NB], f32)
                nc.vector.tensor_reduce(
                    out=mx[:], in_=vcol[:].rearrange("p (b c) -> p b c", b=NB),
                    axis=mybir.AxisListType.X, op=mybir.AluOpType.max)
                u = pf.tile([P, NB * CLS], f32)
                nc.vector.tensor_tensor(
                    out=u[:].rearrange("p (b c) -> p b c", b=NB),
                    in0=vcol[:].rearrange("p (b c) -> p b c", b=NB),
                    in1=_sub(mx[:], 0, [[1, NB], [0, CLS]]),
                    op=mybir.AluOpType.subtract)
                nc.scalar.activation(out=u[:], in_=u[:], func=EXP)
                sm = pf.tile([P, NB], f32)
                nc.vector.tensor_reduce(
                    out=sm[:], in_=u[:].rearrange("p (b c) -> p b c", b=NB),
                    axis=mybir.AxisListType.X, op=mybir.AluOpType.add)
                ls = pf.tile([P, NB], f32)
                nc.scalar.activation(out=ls[:], in_=sm[:], func=LN)
                nc.vector.tensor_tensor(out=ls[:], in0=ls[:], in1=mx[:],
                                        op=mybir.AluOpType.add)
                res = pf.tile([P, NB * CLS], f32)
                nc.vector.tensor_tensor(
                    out=res[:].rearrange("p (b c) -> p b c", b=NB),
                    in0=vcol[:].rearrange("p (b c) -> p b c", b=NB),
                    in1=_sub(ls[:], 0, [[1, NB], [0, CLS]]),
                    op=mybir.AluOpType.subtract)
                nc.sync.dma_start(out=out_d[:, :], in_=res[:])
    nc.finalize()
    return nc


def install_ntff_hook(so_path="/opt/axon/libaxon_pjrt.so"):
    import types
    import ctypes
    import contextlib
    import antenv

    if getattr(antenv, "axon_hooks", None) is not None:
        return
    lib = ctypes.CDLL(so_path)
    if not hasattr(lib, "axon_start_nrt_profile"):
        return
    lib.axon_start_nrt_profile.argtypes = [ctypes.POINTER(ctypes.c_int64),
                                           ctypes.c_size_t]
    lib.axon_start_nrt_profile.restype = ctypes.c_int64
    lib.axon_stop_nrt_profile.argtypes = [ctypes.c_char_p]
    lib.axon_stop_nrt_profile.restype = ctypes.c_int64

    @contextlib.contextmanager
    def _hook(output_dir, device_ids):
        import jax
        jax.devices()
        if device_ids:
            ids = (ctypes.c_int64 * len(device_ids))(*device_ids)
            rc = lib.axon_start_nrt_profile(ids, len(device_ids))
        else:
            rc = lib.axon_start_nrt_profile(None, 0)
        if rc != 0:
            raise RuntimeError(f"axon_start_nrt_profile rc={rc}")
        try:
            yield
        finally:
            n = lib.axon_stop_nrt_profile(str(output_dir).encode())
            print(f"ntff profile: {n} file(s) written to {output_dir}")

    mod = types.ModuleType("antenv.axon_hooks")
    _reg = [_hook]
    mod.set_axon_ntff_profile_hook = lambda h: _reg.__setitem__(0, h)
    mod.get_axon_ntff_profile_hook = lambda: _reg[0]
    sys.modules["antenv.axon_hooks"] = mod
    antenv.axon_hooks = mod


def run(inputs, cfg, trace=False, **kwargs):
    if trace:
        install_ntff_hook()
    in_maps, meta = prep(inputs, cfg)
    nc = build(meta)
    res = bass_utils.run_bass_kernel_spmd(
        nc, in_maps, core_ids=list(range(cfg["NC"])), trace=trace, **kwargs)
    NB, NPC, CLS = meta["NB"], meta["NPC"], meta["CLS"]
    outs = []
    for c in range(cfg["NC"]):
        r = np.asarray(res.results[c]["out"])          # [P, NB*CLS]
        r = r.reshape(P, NB, CLS).transpose(1, 0, 2).reshape(NB * P, CLS)
        outs.append(r[:NPC])
    return np.concatenate(outs, axis=0), res


# ----------------------------------------------------------------------------
# harness entry point
# ----------------------------------------------------------------------------

_CFG = dict(N=100000, F=165, H=4, C=64, CLS=2, NC=8)


def kernel(**inputs):
    """Full (unsharded) inputs -> full [N, 2] float32 log-softmax output.

    Shards edges by destination-node range across the 8 NeuronCores,
    compiles and runs the Bass/Tile kernel via run_bass_kernel_spmd,
    and concatenates the per-core output slices.
    """
    out, _ = run(inputs, _CFG, trace=False)
    return np.ascontiguousarray(out.astype(np.float32))
